# revision 1
# baseline (speedup 1.0000x reference)
"""Trainium2 Bass kernel for segment-packed sliding-window linear attention
(ELU+1 feature map), sharded one head per NeuronCore (8 heads / 8 cores).

Math (per head, per position t):
    qf = elu(q*0.125)+1, kf = elu(k)+1, b(t) = max(seg_start(t), t-1024)
    out[t] = qf_t @ (KV[t]-KVpad[b(t)]) / max(qf_t @ (K[t]-Kpad[b(t)]), eps)
with KV/K *global* causal cumsums of kf (outer) vaug.  Chunked at C=128:
  A  = (Qf Kf_i^T (*) tri<=) Vaug_i + Qf @ S[i]
  B  = active:  (Qf Kf_{i-8}^T (*) tri<) Vaug_{i-8} + Qf @ S[i-8]
       else:    Qf @ P[seg_id]          (prefix matrix per boundary)
  num|den = A - B    (den rides along as Vaug's 65th "ones" column)
All seqlens-dependent control (chunk classification, blend vectors, prefix
masks) is computed host-side and baked into the traced program / tiny aux
inputs.  dens are extracted with two strided batched PSUM reads per pair of
banks.
"""

import numpy as np

import concourse.bass as bass
import concourse.mybir as mybir
import concourse.tile as tile
from concourse.bass_utils import run_bass_kernel_spmd

T, H, D = 4096, 8, 64
C = 128                 # chunk length (partition dim)
NCH = T // C            # 32 chunks
WIN = 1024
WCH = WIN // C          # window = 8 chunks back
M1 = D + 1              # V augmented with ones column -> den for free
SCALE = 0.125
EPS = 1e-6
F32 = mybir.dt.float32
F16 = mybir.dt.float16

TRACE = False           # test harness can flip for NTFF profiling
ALU = mybir.AluOpType
AF = mybir.ActivationFunctionType


# ----------------------------------------------------------------- host plan
def host_plan(seqlens):
    s = np.asarray(seqlens).astype(np.int64)
    assert s.shape[0] >= 2
    pos = np.arange(T)
    seg_id = np.searchsorted(s[1:], pos, side="left")       # [T]
    seg_start = s[seg_id]
    active = seg_start < pos - WIN
    nb = s.shape[0]

    chunks = []
    for i in range(NCH):
        sl = slice(i * C, (i + 1) * C)
        act = active[sl]
        sids = np.unique(seg_id[sl][~act]) if (~act).any() else np.array([], np.int64)
        if act.all():
            chunks.append(dict(kind="W"))
        elif not act.any() and len(sids) == 1:
            chunks.append(dict(kind="S", sid=int(sids[0])))
        else:
            groups = [(int(sid),
                       ((~act) & (seg_id[sl] == sid)).astype(np.float32))
                      for sid in sids]
            chunks.append(dict(kind="G", alpha=act.astype(np.float32),
                               groups=groups))
    bneed = []
    for kc in range(NCH):
        qc = kc + WCH
        if qc >= NCH:
            bneed.append(None)
        else:
            ch = chunks[qc]
            if ch["kind"] == "W":
                bneed.append("neg")
            elif ch["kind"] == "G" and ch["alpha"].any():
                bneed.append("pos")
            else:
                bneed.append(None)
    bnds = []
    for j in range(nb):
        bj = int(np.clip(s[j], 0, T))
        bnds.append((bj // C, bj % C))
    return dict(chunks=chunks, bneed=bneed, bnds=bnds, nb=nb)


def build_aux(plan):
    tri_a = np.triu(np.ones((C, C), np.float32))            # [sl, tl] sl<=tl
    tri_s = np.triu(np.ones((C, C), np.float32), k=1)       # sl<tl
    masks = np.zeros((C, 512), np.float32)
    masks[:, 0:128] = tri_a
    masks[:, 128:256] = -tri_s
    masks[:, 256:384] = tri_a
    masks[:, 384:512] = tri_s

    nb = plan["nb"]
    pmask = np.zeros((C, nb), np.float32)
    for j, (cb, rb) in enumerate(plan["bnds"]):
        pmask[:, j] = (np.arange(C) < rb).astype(np.float32)

    negalpha = np.zeros((C, NCH), np.float32)
    negbeta = np.zeros((C, NCH * nb), np.float32)
    for i, ch in enumerate(plan["chunks"]):
        if ch["kind"] == "G":
            negalpha[:, i] = -ch["alpha"]
            for sid, beta in ch["groups"]:
                negbeta[:, i * nb + sid] = -beta
    return masks, pmask, negalpha, negbeta


def pack_head(q, k, v):
    """q,k,v: [T, D] fp32 one head -> device layouts."""
    qtp = q.T                            # [64, 4096]
    ktp = k.T
    kn = k.reshape(NCH, C, D).transpose(1, 0, 2).reshape(C, NCH * D)
    va = np.concatenate([v.reshape(NCH, C, D),
                         np.ones((NCH, C, 1), np.float32)], axis=2)
    vaug = va.transpose(1, 0, 2).reshape(C, NCH * M1).astype(np.float16)
    return (np.ascontiguousarray(qtp), np.ascontiguousarray(ktp),
            np.ascontiguousarray(kn), np.ascontiguousarray(vaug))


# ------------------------------------------------------------- bass program
def build_bass(plan):
    nb = plan["nb"]
    nc = bass.Bass()
    d_qtp = nc.dram_tensor("qtp", [D, T], F32, kind="ExternalInput")
    d_ktp = nc.dram_tensor("ktp", [D, T], F32, kind="ExternalInput")
    d_kn = nc.dram_tensor("kn", [C, NCH * D], F32, kind="ExternalInput")
    d_vaug = nc.dram_tensor("vaug", [C, NCH * M1], F16,
                            kind="ExternalInput")
    d_masks = nc.dram_tensor("masks", [C, 512], F16, kind="ExternalInput")
    d_pmask = nc.dram_tensor("pmask", [C, nb], F32, kind="ExternalInput")
    d_nalpha = nc.dram_tensor("negalpha", [C, NCH], F32, kind="ExternalInput")
    d_nbeta = nc.dram_tensor("negbeta", [C, NCH * nb], F32,
                             kind="ExternalInput")
    d_out = nc.dram_tensor("out", [T, D], F32, kind="ExternalOutput")

    def tchunk(t, j):
        """[64,128] slice of a transposed [64, T] tensor, chunk j."""
        return t[:, C * j:C * (j + 1)]

    def kchunk(t, c):
        return t[:, c * D:(c + 1) * D]

    def vchunk(t, c):
        return t[:, c * M1:(c + 1) * M1]

    def sslot(t, c):
        return t[:, c * M1:(c + 1) * M1]

    with tile.TileContext(nc) as tc:
        with (
            tc.tile_pool(name="persist", bufs=1) as pp,
            tc.tile_pool(name="stm", bufs=10) as stm_pool,
            tc.tile_pool(name="outp", bufs=8) as out_pool,
            tc.tile_pool(name="pst", bufs=2, space="PSUM") as pst,
            tc.tile_pool(name="pbig", bufs=1, space="PSUM") as pbig,
            tc.tile_pool(name="tmps", bufs=4) as tmp_pool,
        ):
            qtp = pp.tile([D, T], F32)
            ktp = pp.tile([D, T], F32)
            kn = pp.tile([C, NCH * D], F32)
            vaug = pp.tile([C, NCH * M1], F16)
            masks = pp.tile([C, 512], F16)
            pmaskt = pp.tile([C, nb], F32)
            nalpha = pp.tile([C, NCH], F32)
            nbeta = pp.tile([C, NCH * nb], F32)
            sall = pp.tile([D, (NCH + 1) * M1], F16)
            call = pp.tile([D, NCH * M1], F16)
            pall = pp.tile([D, nb * M1], F16)
            rall = pp.tile([C, 42], F32)
            dmax = pp.tile([C, 42], F32)
            e_q = pp.tile([D, T], F16)
            e_k = pp.tile([D, T], F16)
            e_kn = pp.tile([C, NCH * D], F16)
            r_q = pp.tile([D, T], F16)
            r_k = pp.tile([D, T], F16)
            r_kn = pp.tile([C, NCH * D], F16)

            # num slots: one 5-bank PSUM tensor, slot i at 512*(i//7)+65*(i%7)
            pnum = pbig.tile([C, 2560], F32)
            st_ps = pbig.tile([D, M1], F32)   # pass-1 running state

            def num_slot(i):
                off = 512 * (i // 7) + M1 * (i % 7)
                return pnum[:, off:off + M1]

            dma = nc.default_dma_engine
            # kn first: pass-1 and the feat pipeline consume it earliest
            nc.scalar.dma_start(out=kn, in_=d_kn[:, :])
            nc.sync.dma_start(out=qtp, in_=d_qtp[:, :])
            nc.scalar.dma_start(out=ktp, in_=d_ktp[:, :])
            nc.sync.dma_start(out=vaug, in_=d_vaug[:, :])
            nc.scalar.dma_start(out=masks, in_=d_masks[:, :])
            nc.sync.dma_start(out=pmaskt, in_=d_pmask[:, :])
            nc.sync.dma_start(out=nalpha, in_=d_nalpha[:, :])
            nc.sync.dma_start(out=nbeta, in_=d_nbeta[:, :])

            # ---- features: feat(x) = min(exp(s*x),1) + max(s*x,0)
            # kn first (pass-1 consumes it); exp on ACT, relu + fused
            # min/add combine on DVE
            nc.scalar.activation(e_kn, kn, AF.Exp, scale=1.0)
            nc.vector.tensor_scalar(r_kn, kn, 0.0, None, ALU.max)
            nc.vector.scalar_tensor_tensor(e_kn, e_kn, 1.0, r_kn,
                                           ALU.min, ALU.add)
            nc.scalar.activation(e_k, ktp, AF.Exp, scale=1.0)
            nc.vector.tensor_scalar(r_k, ktp, 0.0, None, ALU.max)
            nc.vector.scalar_tensor_tensor(e_k, e_k, 1.0, r_k,
                                           ALU.min, ALU.add)
            nc.scalar.activation(e_q, qtp, AF.Exp, scale=SCALE)
            nc.vector.tensor_scalar(r_q, qtp, 0.0, SCALE, ALU.max, ALU.mult)
            nc.vector.scalar_tensor_tensor(e_q, e_q, 1.0, r_q,
                                           ALU.min, ALU.add)
            qtf, ktf, kf = e_q, e_k, e_kn

            # ---- pass 1: chunk states, running in PSUM, snapshots to SBUF
            nc.vector.memset(sall[:, 0:M1], 0.0)
            for c in range(NCH):
                nc.tensor.matmul(st_ps, lhsT=kchunk(kf, c),
                                 rhs=vchunk(vaug, c),
                                 start=(c == 0), stop=(c == NCH - 1))
                nc.scalar.copy(sslot(sall, c + 1), st_ps)

            # ---- boundary prefix matrices P[j] = cumsum over [0, s_j)
            for j, (cb, rb) in enumerate(plan["bnds"]):
                dst = sslot(pall, j)
                if cb >= NCH:
                    nc.vector.tensor_copy(dst, sslot(sall, NCH))
                elif rb == 0:
                    nc.vector.tensor_copy(dst, sslot(sall, cb))
                else:
                    km = tmp_pool.tile([C, D], F16, tag="km",
                                       name=f"km{j}")
                    nc.vector.tensor_scalar_mul(km, kchunk(kf, cb),
                                                pmaskt[:, j:j + 1])
                    pps = pst.tile([D, M1], F32, tag="st", name=f"pps{j}")
                    nc.tensor.matmul(pps, lhsT=km,
                                     rhs=vchunk(vaug, cb),
                                     start=True, stop=True)
                    nc.vector.scalar_tensor_tensor(dst, pps, 0.0,
                                                   sslot(sall, cb),
                                                   ALU.add, ALU.add)

            # ---- C matrices: W runs batched, S chunks individual
            i = 0
            while i < NCH:
                if plan["chunks"][i]["kind"] == "W":
                    j = i
                    while j < NCH and plan["chunks"][j]["kind"] == "W":
                        j += 1
                    for i0 in range(i, j, 4):
                        n = (min(i0 + 4, j) - i0) * M1
                        nc.vector.scalar_tensor_tensor(
                            call[:, i0 * M1:i0 * M1 + n],
                            sall[:, i0 * M1:i0 * M1 + n], -1.0,
                            sall[:, (i0 - WCH) * M1:(i0 - WCH) * M1 + n],
                            ALU.bypass, ALU.subtract)
                    i = j
                else:
                    i += 1
            for i, ch in enumerate(plan["chunks"]):
                if ch["kind"] == "S":
                    nc.vector.scalar_tensor_tensor(
                        sslot(call, i), sslot(sall, i), -1.0,
                        sslot(pall, ch["sid"]),
                        ALU.bypass, ALU.subtract)

            # ---- pass 2, interleaved: scores for kc=i, then accumulate qc=i
            SV_DT = F16
            vsrc = vaug
            qtf_g = qtf.rearrange("p (g c) -> p g c", c=128)

            def sv_cast(ap):
                return ap

            stm_tiles = {}
            pn = pnum.rearrange("p (b s) -> p b s", s=512)
            masks_g = masks.rearrange("p (x c) -> p x c", c=128)

            def emit_scores(i):
                # scores for key chunk kc=i; two consecutive narrow chunks
                # share one PSUM tile + one mask op (halves DVE op count)
                kc = i
                bm = plan["bneed"][kc]
                wide = bm is not None
                nxt = kc + 1
                pair = (not wide and nxt < NCH
                        and plan["bneed"][nxt] is None)
                stp = pst.tile([C, 256], F32, tag="st", name=f"stp{kc}")
                stm = stm_pool.tile([C, 256], SV_DT, tag="stm",
                                    name=f"stm{kc}")
                if wide:
                    rhs = qtf_g[:, kc:kc + WCH + 1:WCH, :]
                    nc.tensor.matmul(stp, lhsT=tchunk(ktf, kc),
                                     rhs=rhs, start=True, stop=True)
                    moff = 256 if bm == "pos" else 0
                    nc.vector.scalar_tensor_tensor(
                        stm, stp, 1.0, masks[:, moff:moff + 256],
                        ALU.bypass, ALU.mult)
                    stm_tiles[kc] = (stm[:, :128], stm[:, 128:256])
                elif pair:
                    for x, c in enumerate((kc, nxt)):
                        nc.tensor.matmul(stp[:, 128 * x:128 * (x + 1)],
                                         lhsT=tchunk(ktf, c),
                                         rhs=tchunk(qtf, c),
                                         start=True, stop=True)
                    nc.vector.scalar_tensor_tensor(
                        stm.rearrange("p (x c) -> p x c", c=128),
                        stp.rearrange("p (x c) -> p x c", c=128), 1.0,
                        masks_g[:, 0:3:2, :], ALU.bypass, ALU.mult)
                    stm_tiles[kc] = (stm[:, :128], None)
                    stm_tiles[nxt] = (stm[:, 128:256], None)
                else:
                    nc.tensor.matmul(stp[:, :128], lhsT=tchunk(ktf, kc),
                                     rhs=tchunk(qtf, kc),
                                     start=True, stop=True)
                    nc.vector.scalar_tensor_tensor(
                        stm[:, :128], stp[:, :128], 1.0, masks[:, 0:128],
                        ALU.bypass, ALU.mult)
                    stm_tiles[kc] = (stm[:, :128], None)

            for i in range(NCH):
                if i not in stm_tiles:
                    emit_scores(i)

                # accumulate num for query chunk qc=i
                ch = plan["chunks"][i]
                slot = num_slot(i)
                kind = ch["kind"]
                nc.tensor.matmul(slot, lhsT=sv_cast(stm_tiles[i][0]),
                                 rhs=sv_cast(vchunk(vsrc, i)),
                                 start=True, stop=False)
                if kind == "W":
                    nc.tensor.matmul(slot,
                                     lhsT=sv_cast(stm_tiles[i - WCH][1]),
                                     rhs=sv_cast(vchunk(vsrc, i - WCH)),
                                     start=False, stop=False)
                    nc.tensor.matmul(slot, lhsT=tchunk(qtf, i),
                                     rhs=sslot(call, i),
                                     start=False, stop=True)
                elif kind == "S":
                    nc.tensor.matmul(slot, lhsT=tchunk(qtf, i),
                                     rhs=sslot(call, i),
                                     start=False, stop=True)
                else:  # G
                    nc.tensor.matmul(slot, lhsT=tchunk(qtf, i),
                                     rhs=sslot(sall, i),
                                     start=False, stop=True)
                    terms = []
                    if ch["alpha"].any():
                        bw = pst.tile([C, M1], F32, tag="st", name=f"bw{i}")
                        nc.tensor.matmul(
                            bw, lhsT=sv_cast(stm_tiles[i - WCH][1]),
                            rhs=sv_cast(vchunk(vsrc, i - WCH)),
                            start=True, stop=False)
                        nc.tensor.matmul(bw, lhsT=tchunk(qtf, i),
                                         rhs=sslot(sall, i - WCH),
                                         start=False, stop=True)
                        terms.append((bw, nalpha[:, i:i + 1]))
                    for sid, _ in ch["groups"]:
                        gp = pst.tile([C, M1], F32, tag="st",
                                      name=f"gp{i}_{sid}")
                        nc.tensor.matmul(gp, lhsT=tchunk(qtf, i),
                                         rhs=sslot(pall, sid),
                                         start=True, stop=True)
                        terms.append((gp, nbeta[:, i * nb + sid:
                                                i * nb + sid + 1]))
                    # fold: slot = main + sum(term * negscale).
                    # DVE reads at most one PSUM operand per op, so move the
                    # main accumulator to SBUF first, then chain terms.
                    acc = tmp_pool.tile([C, M1], F32, tag="gt",
                                        name=f"gacc{i}")
                    nc.scalar.copy(acc, slot)
                    for t_idx, (tps, sc) in enumerate(terms):
                        last = t_idx == len(terms) - 1
                        dst = slot if last else tmp_pool.tile(
                            [C, M1], F32, tag="gt", name=f"gt{i}_{t_idx}")
                        nc.vector.scalar_tensor_tensor(
                            dst, tps, sc, acc, ALU.mult, ALU.add)
                        acc = dst

                # dens for a completed PSUM bank, emitted inline so they
                # run ahead of the remaining mask ops in DVE's queue
                if i % 7 == 6 or i == NCH - 1:
                    g = i // 7
                    dv = pn[:, g, D:D + 65 * 6 + 1:65]
                    sel = slice(7 * g, 7 * g + 7)
                    nc.vector.tensor_scalar_max(dmax[:, sel], dv, EPS)
                    nc.vector.reciprocal(rall[:, sel], dmax[:, sel])

            # ---- scale + store
            for i in range(NCH):
                ob = out_pool.tile([C, D], F32, tag="ob", name=f"ob{i}")
                nc.scalar.activation(ob, num_slot(i)[:, :D], AF.Copy,
                                     scale=rall[:, i:i + 1])
                eng = nc.sync if i % 2 == 0 else nc.scalar
                eng.dma_start(out=d_out[i * C:(i + 1) * C, :], in_=ob)
    return nc


def split_waits(bir: bytes) -> bytes:
    """Walrus codegen caps sync waits at 1 per instruction (2 for
    EventSemaphore); Tile sometimes attaches more.  Hoist the excess into
    preceding same-engine NoOps (engines are in-order, so semantics hold)."""
    import json
    m = json.loads(bir)
    for f in m["functions"]:
        for bb in f["blocks"]:
            out = []
            for ins in bb["instructions"]:
                si = ins.get("sync_info")
                ow = (si or {}).get("on_wait") or []
                cap = 2 if ins.get("opcode") == "EventSemaphore" else 1
                eng = ins.get("engine")
                if eng and len(ow) > cap:
                    keep = ow[-cap:]
                    for j, w in enumerate(ow[:-cap]):
                        out.append({"name": f'{ins["name"]}_sw{j}',
                                    "opcode": "NoOp", "engine": eng,
                                    "ins": [], "outs": [],
                                    "sync_info": {"on_wait": [w],
                                                  "on_update": []}})
                    ins = dict(ins)
                    ins["sync_info"] = {
                        "on_wait": keep,
                        "on_update": (si or {}).get("on_update") or []}
                out.append(ins)
            bb["instructions"] = out
    return json.dumps(m).encode()


# ------------------------------------------------------------------ driver
def kernel(**inputs):
    q = np.ascontiguousarray(np.asarray(inputs["q"]), dtype=np.float32)
    k = np.ascontiguousarray(np.asarray(inputs["k"]), dtype=np.float32)
    v = np.ascontiguousarray(np.asarray(inputs["v"]), dtype=np.float32)
    seqlens = np.asarray(inputs["seqlens"])
    assert q.shape == (T, H, D), q.shape

    plan = host_plan(seqlens)
    masks, pmask, negalpha, negbeta = build_aux(plan)
    nc = build_bass(plan)
    patched = split_waits(nc.to_json_bytes())
    nc.to_json_bytes = lambda: patched

    in_maps = []
    for h in range(H):
        qtp, ktp, kn, vaug = pack_head(q[:, h], k[:, h], v[:, h])
        im = dict(qtp=qtp, ktp=ktp, kn=kn, vaug=vaug,
                  masks=masks.astype(np.float16),
                  pmask=pmask, negalpha=negalpha, negbeta=negbeta)
        in_maps.append(im)

    res = run_bass_kernel_spmd(nc, in_maps, core_ids=list(range(H)),
                               trace=TRACE)
    if TRACE:
        kernel.last_result = res
    out = np.empty((T, H, D), np.float32)
    for h in range(H):
        out[:, h, :] = res.results[h]["out"]
    return out



# revision 6
# speedup vs baseline: 1.2748x; 1.2748x over previous
"""Trainium2 Bass kernel for segment-packed sliding-window linear attention
(ELU+1 feature map), sharded one head per NeuronCore (8 heads / 8 cores).

Math (per head, per position t):
    qf = elu(q*0.125)+1, kf = elu(k)+1, b(t) = max(seg_start(t), t-1024)
    out[t] = qf_t @ (KV[t]-KVpad[b(t)]) / max(qf_t @ (K[t]-Kpad[b(t)]), eps)
with KV/K *global* causal cumsums of kf (outer) vaug.  Chunked at C=128:
  A  = (Qf Kf_i^T (*) tri<=) Vaug_i + Qf @ S[i]
  B  = active:  (Qf Kf_{i-8}^T (*) tri<) Vaug_{i-8} + Qf @ S[i-8]
       else:    Qf @ P[seg_id]          (prefix matrix per boundary)
  num|den = A - B    (den rides along as Vaug's 65th "ones" column)
All seqlens-dependent control (chunk classification, blend vectors, prefix
masks) is computed host-side and baked into the traced program / tiny aux
inputs.  dens are extracted with two strided batched PSUM reads per pair of
banks.
"""

import numpy as np

import concourse.bass as bass
import concourse.mybir as mybir
import concourse.tile as tile
from concourse.bass_utils import run_bass_kernel_spmd

T, H, D = 4096, 8, 64
C = 128                 # chunk length (partition dim)
NCH = T // C            # 32 chunks
WIN = 1024
WCH = WIN // C          # window = 8 chunks back
M1 = D + 1              # V augmented with ones column -> den for free
SCALE = 0.125
EPS = 1e-6
F32 = mybir.dt.float32
F16 = mybir.dt.float16

TRACE = False           # test harness can flip for NTFF profiling
ALU = mybir.AluOpType
AF = mybir.ActivationFunctionType


# ----------------------------------------------------------------- host plan
def host_plan(seqlens):
    s = np.asarray(seqlens).astype(np.int64)
    assert s.shape[0] >= 2
    pos = np.arange(T)
    seg_id = np.searchsorted(s[1:], pos, side="left")       # [T]
    seg_start = s[seg_id]
    active = seg_start < pos - WIN
    nb = s.shape[0]

    chunks = []
    for i in range(NCH):
        sl = slice(i * C, (i + 1) * C)
        act = active[sl]
        sids = np.unique(seg_id[sl][~act]) if (~act).any() else np.array([], np.int64)
        if act.all():
            chunks.append(dict(kind="W"))
        elif not act.any() and len(sids) == 1:
            chunks.append(dict(kind="S", sid=int(sids[0])))
        else:
            groups = [(int(sid),
                       ((~act) & (seg_id[sl] == sid)).astype(np.float32))
                      for sid in sids]
            chunks.append(dict(kind="G", alpha=act.astype(np.float32),
                               groups=groups))
    bneed = []
    for kc in range(NCH):
        qc = kc + WCH
        if qc >= NCH:
            bneed.append(None)
        else:
            ch = chunks[qc]
            if ch["kind"] == "W":
                bneed.append("neg")
            elif ch["kind"] == "G" and ch["alpha"].any():
                bneed.append("pos")
            else:
                bneed.append(None)
    bnds = []
    for j in range(nb):
        bj = int(np.clip(s[j], 0, T))
        bnds.append((bj // C, bj % C))
    return dict(chunks=chunks, bneed=bneed, bnds=bnds, nb=nb)


def build_aux(plan):
    tri_a = np.triu(np.ones((C, C), np.float32))            # [sl, tl] sl<=tl
    tri_s = np.triu(np.ones((C, C), np.float32), k=1)       # sl<tl
    masks = np.zeros((C, 512), np.float32)
    masks[:, 0:128] = tri_a
    masks[:, 128:256] = -tri_s
    masks[:, 256:384] = tri_a
    masks[:, 384:512] = tri_s

    nb = plan["nb"]
    pmask = np.zeros((C, nb), np.float32)
    for j, (cb, rb) in enumerate(plan["bnds"]):
        pmask[:, j] = (np.arange(C) < rb).astype(np.float32)

    negalpha = np.zeros((C, NCH), np.float32)
    negbeta = np.zeros((C, NCH * nb), np.float32)
    for i, ch in enumerate(plan["chunks"]):
        if ch["kind"] == "G":
            negalpha[:, i] = -ch["alpha"]
            for sid, beta in ch["groups"]:
                negbeta[:, i * nb + sid] = -beta
    return masks, pmask, negalpha, negbeta


def pack_head(q, k, v):
    """q,k,v: [T, D] fp32 one head -> device layouts."""
    qtp = q.T                            # [64, 4096]
    ktp = k.T
    kn = k.reshape(NCH, C, D).transpose(1, 0, 2).reshape(C, NCH * D)
    va = np.concatenate([v.reshape(NCH, C, D),
                         np.ones((NCH, C, 1), np.float32)], axis=2)
    vaug = va.transpose(1, 0, 2).reshape(C, NCH * M1).astype(np.float16)
    return (np.ascontiguousarray(qtp), np.ascontiguousarray(ktp),
            np.ascontiguousarray(kn), np.ascontiguousarray(vaug))


# ------------------------------------------------------------- bass program
def build_bass(plan):
    nb = plan["nb"]
    nc = bass.Bass()
    d_qtp = nc.dram_tensor("qtp", [D, T], F32, kind="ExternalInput")
    d_ktp = nc.dram_tensor("ktp", [D, T], F32, kind="ExternalInput")
    d_kn = nc.dram_tensor("kn", [C, NCH * D], F32, kind="ExternalInput")
    d_vaug = nc.dram_tensor("vaug", [C, NCH * M1], F16,
                            kind="ExternalInput")
    d_masks = nc.dram_tensor("masks", [C, 512], F16, kind="ExternalInput")
    d_pmask = nc.dram_tensor("pmask", [C, nb], F32, kind="ExternalInput")
    d_nalpha = nc.dram_tensor("negalpha", [C, NCH], F32, kind="ExternalInput")
    d_nbeta = nc.dram_tensor("negbeta", [C, NCH * nb], F32,
                             kind="ExternalInput")
    d_out = nc.dram_tensor("out", [T, D], F32, kind="ExternalOutput")

    def tchunk(t, j):
        """[64,128] slice of a transposed [64, T] tensor, chunk j."""
        return t[:, C * j:C * (j + 1)]

    def kchunk(t, c):
        return t[:, c * D:(c + 1) * D]

    def vchunk(t, c):
        return t[:, c * M1:(c + 1) * M1]

    def sslot(t, c):
        return t[:, c * M1:(c + 1) * M1]

    with tile.TileContext(nc) as tc:
        with (
            tc.tile_pool(name="persist", bufs=1) as pp,
            tc.tile_pool(name="stm", bufs=10) as stm_pool,
            tc.tile_pool(name="outp", bufs=8) as out_pool,
            tc.tile_pool(name="pst", bufs=2, space="PSUM") as pst,
            tc.tile_pool(name="pbig", bufs=1, space="PSUM") as pbig,
            tc.tile_pool(name="tmps", bufs=4) as tmp_pool,
        ):
            qtp = pp.tile([D, T], F32)
            ktp = pp.tile([D, T], F32)
            kn = pp.tile([C, NCH * D], F32)
            vaug = pp.tile([C, NCH * M1], F16)
            masks = pp.tile([C, 512], F16)
            pmaskt = pp.tile([C, nb], F32)
            nalpha = pp.tile([C, NCH], F32)
            nbeta = pp.tile([C, NCH * nb], F32)
            sall = pp.tile([D, (NCH + 1) * M1], F16)
            call = pp.tile([D, NCH * M1], F16)
            pall = pp.tile([D, nb * M1], F16)
            rall = pp.tile([C, 42], F32)
            dmax = pp.tile([C, 42], F32)
            e_q = pp.tile([D, T], F16)
            e_k = pp.tile([D, T], F16)
            e_kn = pp.tile([C, NCH * D], F16)
            r_q = pp.tile([D, T], F16)
            r_k = pp.tile([D, T], F16)
            r_kn = pp.tile([C, NCH * D], F16)

            # num slots: one 5-bank PSUM tensor, slot i at 512*(i//7)+65*(i%7)
            pnum = pbig.tile([C, 2560], F32)
            st_ps = pbig.tile([D, M1], F32)   # pass-1 running state

            def num_slot(i):
                off = 512 * (i // 7) + M1 * (i % 7)
                return pnum[:, off:off + M1]

            dma = nc.default_dma_engine
            # kn first: pass-1 and the feat pipeline consume it earliest
            nc.scalar.dma_start(out=kn, in_=d_kn[:, :])
            nc.sync.dma_start(out=qtp, in_=d_qtp[:, :])
            nc.scalar.dma_start(out=ktp, in_=d_ktp[:, :])
            nc.sync.dma_start(out=vaug, in_=d_vaug[:, :])
            nc.scalar.dma_start(out=masks, in_=d_masks[:, :])
            nc.sync.dma_start(out=pmaskt, in_=d_pmask[:, :])
            nc.sync.dma_start(out=nalpha, in_=d_nalpha[:, :])
            nc.sync.dma_start(out=nbeta, in_=d_nbeta[:, :])

            # ---- features: feat(x) = min(exp(s*x),1) + max(s*x,0)
            # kn first (pass-1 consumes it); exp on ACT, relu + fused
            # min/add combine on DVE
            nc.scalar.activation(e_kn, kn, AF.Exp, scale=1.0)
            nc.vector.tensor_scalar(r_kn, kn, 0.0, None, ALU.max)
            nc.vector.scalar_tensor_tensor(e_kn, e_kn, 1.0, r_kn,
                                           ALU.min, ALU.add)
            nc.scalar.activation(e_k, ktp, AF.Exp, scale=1.0)
            nc.vector.tensor_scalar(r_k, ktp, 0.0, None, ALU.max)
            nc.vector.scalar_tensor_tensor(e_k, e_k, 1.0, r_k,
                                           ALU.min, ALU.add)
            nc.scalar.activation(e_q, qtp, AF.Exp, scale=SCALE)
            nc.vector.tensor_scalar(r_q, qtp, 0.0, SCALE, ALU.max, ALU.mult)
            nc.vector.scalar_tensor_tensor(e_q, e_q, 1.0, r_q,
                                           ALU.min, ALU.add)
            qtf, ktf, kf = e_q, e_k, e_kn

            # ---- pass 1: chunk states, running in PSUM, snapshots to SBUF
            nc.vector.memset(sall[:, 0:M1], 0.0)
            for c in range(NCH):
                nc.tensor.matmul(st_ps, lhsT=kchunk(kf, c),
                                 rhs=vchunk(vaug, c),
                                 start=(c == 0), stop=(c == NCH - 1))
                nc.scalar.copy(sslot(sall, c + 1), st_ps)

            # ---- boundary prefix matrices P[j] = cumsum over [0, s_j)
            for j, (cb, rb) in enumerate(plan["bnds"]):
                dst = sslot(pall, j)
                if cb >= NCH:
                    nc.vector.tensor_copy(dst, sslot(sall, NCH))
                elif rb == 0:
                    nc.vector.tensor_copy(dst, sslot(sall, cb))
                else:
                    km = tmp_pool.tile([C, D], F16, tag="km",
                                       name=f"km{j}")
                    nc.vector.tensor_scalar_mul(km, kchunk(kf, cb),
                                                pmaskt[:, j:j + 1])
                    pps = pst.tile([D, M1], F32, tag="st", name=f"pps{j}")
                    nc.tensor.matmul(pps, lhsT=km,
                                     rhs=vchunk(vaug, cb),
                                     start=True, stop=True)
                    nc.vector.scalar_tensor_tensor(dst, pps, 0.0,
                                                   sslot(sall, cb),
                                                   ALU.add, ALU.add)

            # ---- C matrices: W runs batched, S chunks individual
            i = 0
            while i < NCH:
                if plan["chunks"][i]["kind"] == "W":
                    j = i
                    while j < NCH and plan["chunks"][j]["kind"] == "W":
                        j += 1
                    for i0 in range(i, j, 4):
                        n = (min(i0 + 4, j) - i0) * M1
                        nc.vector.scalar_tensor_tensor(
                            call[:, i0 * M1:i0 * M1 + n],
                            sall[:, i0 * M1:i0 * M1 + n], -1.0,
                            sall[:, (i0 - WCH) * M1:(i0 - WCH) * M1 + n],
                            ALU.bypass, ALU.subtract)
                    i = j
                else:
                    i += 1
            for i, ch in enumerate(plan["chunks"]):
                if ch["kind"] == "S":
                    nc.vector.scalar_tensor_tensor(
                        sslot(call, i), sslot(sall, i), -1.0,
                        sslot(pall, ch["sid"]),
                        ALU.bypass, ALU.subtract)

            # ---- pass 2, interleaved: scores for kc=i, then accumulate qc=i
            SV_DT = F16
            vsrc = vaug
            qtf_g = qtf.rearrange("p (g c) -> p g c", c=128)

            def sv_cast(ap):
                return ap

            stm_tiles = {}
            pn = pnum.rearrange("p (b s) -> p b s", s=512)
            masks_g = masks.rearrange("p (x c) -> p x c", c=128)

            def emit_scores(i):
                # scores for key chunk kc=i; two consecutive narrow chunks
                # share one PSUM tile + one mask op (halves DVE op count)
                kc = i
                bm = plan["bneed"][kc]
                wide = bm is not None
                nxt = kc + 1
                pair = (not wide and nxt < NCH
                        and plan["bneed"][nxt] is None)
                stp = pst.tile([C, 256], F32, tag="st", name=f"stp{kc}")
                stm = stm_pool.tile([C, 256], SV_DT, tag="stm",
                                    name=f"stm{kc}")
                if wide:
                    rhs = qtf_g[:, kc:kc + WCH + 1:WCH, :]
                    nc.tensor.matmul(stp, lhsT=tchunk(ktf, kc),
                                     rhs=rhs, start=True, stop=True)
                    moff = 256 if bm == "pos" else 0
                    nc.vector.scalar_tensor_tensor(
                        stm, stp, 1.0, masks[:, moff:moff + 256],
                        ALU.bypass, ALU.mult)
                    stm_tiles[kc] = (stm[:, :128], stm[:, 128:256])
                elif pair:
                    for x, c in enumerate((kc, nxt)):
                        nc.tensor.matmul(stp[:, 128 * x:128 * (x + 1)],
                                         lhsT=tchunk(ktf, c),
                                         rhs=tchunk(qtf, c),
                                         start=True, stop=True)
                    nc.vector.scalar_tensor_tensor(
                        stm.rearrange("p (x c) -> p x c", c=128),
                        stp.rearrange("p (x c) -> p x c", c=128), 1.0,
                        masks_g[:, 0:3:2, :], ALU.bypass, ALU.mult)
                    stm_tiles[kc] = (stm[:, :128], None)
                    stm_tiles[nxt] = (stm[:, 128:256], None)
                else:
                    nc.tensor.matmul(stp[:, :128], lhsT=tchunk(ktf, kc),
                                     rhs=tchunk(qtf, kc),
                                     start=True, stop=True)
                    nc.vector.scalar_tensor_tensor(
                        stm[:, :128], stp[:, :128], 1.0, masks[:, 0:128],
                        ALU.bypass, ALU.mult)
                    stm_tiles[kc] = (stm[:, :128], None)

            for i in range(NCH):
                if i not in stm_tiles:
                    emit_scores(i)

                # accumulate num for query chunk qc=i
                ch = plan["chunks"][i]
                slot = num_slot(i)
                kind = ch["kind"]
                nc.tensor.matmul(slot, lhsT=sv_cast(stm_tiles[i][0]),
                                 rhs=sv_cast(vchunk(vsrc, i)),
                                 start=True, stop=False)
                if kind == "W":
                    nc.tensor.matmul(slot,
                                     lhsT=sv_cast(stm_tiles[i - WCH][1]),
                                     rhs=sv_cast(vchunk(vsrc, i - WCH)),
                                     start=False, stop=False)
                    nc.tensor.matmul(slot, lhsT=tchunk(qtf, i),
                                     rhs=sslot(call, i),
                                     start=False, stop=True)
                elif kind == "S":
                    nc.tensor.matmul(slot, lhsT=tchunk(qtf, i),
                                     rhs=sslot(call, i),
                                     start=False, stop=True)
                else:  # G
                    nc.tensor.matmul(slot, lhsT=tchunk(qtf, i),
                                     rhs=sslot(sall, i),
                                     start=False, stop=True)
                    terms = []
                    if ch["alpha"].any():
                        bw = pst.tile([C, M1], F32, tag="st", name=f"bw{i}")
                        nc.tensor.matmul(
                            bw, lhsT=sv_cast(stm_tiles[i - WCH][1]),
                            rhs=sv_cast(vchunk(vsrc, i - WCH)),
                            start=True, stop=False)
                        nc.tensor.matmul(bw, lhsT=tchunk(qtf, i),
                                         rhs=sslot(sall, i - WCH),
                                         start=False, stop=True)
                        terms.append((bw, nalpha[:, i:i + 1]))
                    for sid, _ in ch["groups"]:
                        gp = pst.tile([C, M1], F32, tag="st",
                                      name=f"gp{i}_{sid}")
                        nc.tensor.matmul(gp, lhsT=tchunk(qtf, i),
                                         rhs=sslot(pall, sid),
                                         start=True, stop=True)
                        terms.append((gp, nbeta[:, i * nb + sid:
                                                i * nb + sid + 1]))
                    # fold: slot = main + sum(term * negscale).
                    # DVE reads at most one PSUM operand per op, so move the
                    # main accumulator to SBUF first, then chain terms.
                    acc = tmp_pool.tile([C, M1], F32, tag="gt",
                                        name=f"gacc{i}")
                    nc.scalar.copy(acc, slot)
                    for t_idx, (tps, sc) in enumerate(terms):
                        last = t_idx == len(terms) - 1
                        dst = slot if last else tmp_pool.tile(
                            [C, M1], F32, tag="gt", name=f"gt{i}_{t_idx}")
                        nc.vector.scalar_tensor_tensor(
                            dst, tps, sc, acc, ALU.mult, ALU.add)
                        acc = dst

                # dens for a completed PSUM bank, emitted inline so they
                # run ahead of the remaining mask ops in DVE's queue
                if i % 7 == 6 or i == NCH - 1:
                    g = i // 7
                    dv = pn[:, g, D:D + 65 * 6 + 1:65]
                    sel = slice(7 * g, 7 * g + 7)
                    nc.vector.tensor_scalar_max(dmax[:, sel], dv, EPS)
                    nc.vector.reciprocal(rall[:, sel], dmax[:, sel])

            # ---- scale + store
            for i in range(NCH):
                ob = out_pool.tile([C, D], F32, tag="ob", name=f"ob{i}")
                nc.scalar.activation(ob, num_slot(i)[:, :D], AF.Copy,
                                     scale=rall[:, i:i + 1])
                eng = nc.sync if i % 2 == 0 else nc.scalar
                eng.dma_start(out=d_out[i * C:(i + 1) * C, :], in_=ob)
    return nc


def split_waits(bir: bytes) -> bytes:
    """Walrus codegen caps sync waits at 1 per instruction (2 for
    EventSemaphore); Tile sometimes attaches more.  Hoist the excess into
    preceding same-engine NoOps (engines are in-order, so semantics hold)."""
    import json
    m = json.loads(bir)
    for f in m["functions"]:
        for bb in f["blocks"]:
            out = []
            for ins in bb["instructions"]:
                si = ins.get("sync_info")
                ow = (si or {}).get("on_wait") or []
                cap = 2 if ins.get("opcode") == "EventSemaphore" else 1
                eng = ins.get("engine")
                if eng and len(ow) > cap:
                    keep = ow[-cap:]
                    for j, w in enumerate(ow[:-cap]):
                        out.append({"name": f'{ins["name"]}_sw{j}',
                                    "opcode": "NoOp", "engine": eng,
                                    "ins": [], "outs": [],
                                    "sync_info": {"on_wait": [w],
                                                  "on_update": []}})
                    ins = dict(ins)
                    ins["sync_info"] = {
                        "on_wait": keep,
                        "on_update": (si or {}).get("on_update") or []}
                out.append(ins)
            bb["instructions"] = out
    return json.dumps(m).encode()


# ===================================================== fast path
TH = T // 2             # 2048, packed free dim


def fast_plan(seqlens, win):
    """Return list of segment chunk-ranges if the fast path applies, else None."""
    s = np.asarray(seqlens).astype(np.int64)
    s = np.clip(s, 0, T)
    b = np.unique(np.concatenate([[0], s, [T]]))
    if b[0] != 0 or b[-1] != T:
        return None
    if (b % C).any():
        return None
    segs = []
    for a, e in zip(b[:-1], b[1:]):
        if e - a > win:          # sliding window would activate
            return None
        ca, ce = int(a) // C, int(e) // C
        if ca < NCH // 2 < ce:
            return None          # segment straddles the packing boundary
        segs.append((ca, ce))
    return segs


def pack_head_fast(q, k, v, segs):
    """q,k,v: [T, D] fp32 -> (in1, k2, q2, qx) device layouts (f16)."""
    kn = k.reshape(NCH, C, D).transpose(1, 0, 2).reshape(C, NCH * D)
    va = np.concatenate([v.reshape(NCH, C, D),
                         np.ones((NCH, C, 1), np.float32)], axis=2)
    vaug = va.transpose(1, 0, 2).reshape(C, NCH * M1)
    in1 = np.concatenate([kn, vaug], axis=1).astype(np.float16)
    k2 = k.T.reshape(D, 2, TH).transpose(1, 0, 2).reshape(2 * D, TH)
    q2 = q.T.reshape(D, 2, TH).transpose(1, 0, 2).reshape(2 * D, TH)
    # boundary-query columns: query at t = e*C attends over the whole
    # previous segment (searchsorted side='left' semantics); place its raw
    # q column at the previous segment's partition half.
    bnds = [(a, e) for a, e in segs if e < NCH]
    qx = np.zeros((C, max(1, len(bnds))), np.float32)
    for j, (a, e) in enumerate(bnds):
        hb = D if a >= NCH // 2 else 0
        qx[hb:hb + D, j] = q[e * C, :]
    return (np.ascontiguousarray(in1),
            np.ascontiguousarray(k2.astype(np.float16)),
            np.ascontiguousarray(q2.astype(np.float16)),
            np.ascontiguousarray(qx.astype(np.float16)))


def unpack_out_fast(res):
    """[128, NCH*64] f16 chunk-major -> [T, D] f32"""
    o = np.asarray(res, np.float32).reshape(C, NCH, D)
    return o.transpose(1, 0, 2).reshape(T, D)


def build_bass_fast(segs):
    bnds = [(a, e) for a, e in segs if e < NCH]
    NB = max(1, len(bnds))
    nc = bass.Bass()
    d_in1 = nc.dram_tensor("in1", [C, NCH * D + NCH * M1], F16,
                           kind="ExternalInput")
    d_k2 = nc.dram_tensor("k2", [C, TH], F16, kind="ExternalInput")
    d_q2 = nc.dram_tensor("q2", [C, TH], F16, kind="ExternalInput")
    d_qx = nc.dram_tensor("qx", [C, NB], F16, kind="ExternalInput")
    d_masks = nc.dram_tensor("masks", [C, 512], F16, kind="ExternalInput")
    d_out = nc.dram_tensor("out", [C, NCH * D], F16, kind="ExternalOutput")

    seg_start = set(a for a, e in segs)
    bnd_of = {e: (j, a) for j, (a, e) in enumerate(bnds)}

    def tchunk(t, j):
        """[64,128] chunk j of a [128, 2048]-packed transposed tensor."""
        h, r = divmod(j, NCH // 2)
        return t[h * D:(h + 1) * D, r * C:(r + 1) * C]

    def kchunk(t, c):
        return t[:, c * D:(c + 1) * D]

    with tile.TileContext(nc) as tc:
        with (
            tc.tile_pool(name="persist", bufs=1) as pp,
            tc.tile_pool(name="stm", bufs=6) as stm_pool,
            tc.tile_pool(name="pst", bufs=2, space="PSUM") as pst,
            tc.tile_pool(name="pbig", bufs=1, space="PSUM") as pbig,
        ):
            in1 = pp.tile([C, NCH * D + NCH * M1], F16)
            kn = in1[:, :NCH * D]
            vaug = in1[:, NCH * D:]
            k2 = pp.tile([C, TH], F16)
            q2 = pp.tile([C, TH], F16)
            kf = pp.tile([C, NCH * D], F16)      # featured kn
            ktf = pp.tile([C, TH], F16)
            qtf = pp.tile([C, TH], F16)
            rel = pp.tile([C, TH], F16)          # relu scratch for k2/q2
            relk = pp.tile([C, NCH * D], F16)
            masks = pp.tile([C, 512], F16)
            sall = pp.tile([C, NCH * M1 // 2], F16)  # states, 2-half packed
            rall = pp.tile([C, NCH], F32)
            dmax = pp.tile([C, NCH], F32)
            outall = pp.tile([C, NCH * D], F16)
            qx = pp.tile([C, NB], F16)
            qxf = pp.tile([C, NB], F16)
            qxr = pp.tile([C, NB], F16)
            ps3 = pp.tile([C, NB * M1], F16)
            tiny = pp.tile([1, 1], F32)

            pnum = pbig.tile([C, 2560], F32)     # 5 banks; 7 slots/bank

            def num_slot(i):
                off = 512 * (i // 7) + M1 * (i % 7)
                return pnum[:, off:off + M1]

            def sslot(c):
                h, r = divmod(c, NCH // 2)
                return sall[h * D:(h + 1) * D, r * M1:(r + 1) * M1]

            def gslot(c):
                h = c // (NCH // 2)
                return num_slot(c)[h * D:(h + 1) * D, :]

            def vchunk(c):
                return vaug[:, c * M1:(c + 1) * M1]

            # --- warm the ACT exp table before any data arrives
            nc.gpsimd.memset(tiny, 0.0)
            nc.scalar.activation(tiny, tiny, AF.Exp, scale=1.0)

            # --- input DMAs (sync ring, in pipeline order); aux on scalar
            nc.scalar.dma_start(out=masks, in_=d_masks[:, :])
            nc.sync.dma_start(out=in1, in_=d_in1[:, :])
            nc.sync.dma_start(out=k2, in_=d_k2[:, :])
            nc.sync.dma_start(out=q2, in_=d_q2[:, :])
            nc.scalar.dma_start(out=qx, in_=d_qx[:, :])

            # --- features: f(x) = min(exp(s x),1) + relu(s x)
            nc.scalar.activation(kf, kn, AF.Exp, scale=1.0)
            nc.scalar.activation(relk, kn, AF.Relu, scale=1.0)
            nc.vector.scalar_tensor_tensor(kf, kf, 1.0, relk,
                                           ALU.min, ALU.add)
            nc.scalar.activation(ktf, k2, AF.Exp, scale=1.0)
            nc.scalar.activation(rel, k2, AF.Relu, scale=1.0)
            nc.vector.scalar_tensor_tensor(ktf, ktf, 1.0, rel,
                                           ALU.min, ALU.add)
            nc.scalar.activation(qtf, q2, AF.Exp, scale=SCALE)
            nc.scalar.activation(rel, q2, AF.Relu, scale=SCALE)
            nc.vector.scalar_tensor_tensor(qtf, qtf, 1.0, rel,
                                           ALU.min, ALU.add)
            nc.scalar.activation(qxf, qx, AF.Exp, scale=SCALE)
            nc.scalar.activation(qxr, qx, AF.Relu, scale=SCALE)
            nc.vector.scalar_tensor_tensor(qxf, qxf, 1.0, qxr,
                                           ALU.min, ALU.add)

            # --- pass 1: G_c = kf_c^T @ vaug_c for every chunk whose state
            # is consumed, each into its own PSUM slot (no PE serialization)
            used = [c for c in range(NCH - 1)
                    if (c + 1 not in seg_start) or (c + 1 in bnd_of)]
            for c in used:
                nc.tensor.matmul(gslot(c), lhsT=kchunk(kf, c),
                                 rhs=vchunk(c), start=True, stop=True)

            # --- prefix scan (exclusive, segment-local) on DVE
            for a, e in segs:
                for c in range(a + 1, e):
                    if c == a + 1:
                        nc.vector.tensor_copy(sslot(c), gslot(c - 1))
                    else:
                        nc.vector.scalar_tensor_tensor(
                            sslot(c), gslot(c - 1), 0.0,
                            sslot(c - 1), ALU.bypass, ALU.add)
                if e in bnd_of:
                    j, _ = bnd_of[e]
                    hb = D * (a >= NCH // 2)
                    dst = ps3[hb:hb + D, j * M1:(j + 1) * M1]
                    if e - 1 == a:
                        nc.vector.tensor_copy(dst, gslot(e - 1))
                    else:
                        nc.vector.scalar_tensor_tensor(
                            dst, gslot(e - 1), 0.0,
                            sslot(e - 1), ALU.bypass, ALU.add)

            # --- pass 2
            masks_g = masks.rearrange("p (x c) -> p x c", c=128)
            pn = pnum.rearrange("p (b s) -> p b s", s=512)
            stm_tiles = {}
            shipped = [0]

            def emit_scores(c):
                nxt = c + 1
                stp = pst.tile([C, 256], F32, tag="st", name=f"stp{c}")
                stm = stm_pool.tile([C, 256], F16, tag="stm",
                                    name=f"stm{c}")
                if nxt < NCH:
                    for x, cc in enumerate((c, nxt)):
                        nc.tensor.matmul(stp[:, 128 * x:128 * (x + 1)],
                                         lhsT=tchunk(ktf, cc),
                                         rhs=tchunk(qtf, cc),
                                         start=True, stop=True)
                    nc.vector.scalar_tensor_tensor(
                        stm.rearrange("p (x c) -> p x c", c=128),
                        stp.rearrange("p (x c) -> p x c", c=128), 1.0,
                        masks_g[:, 0:3:2, :], ALU.bypass, ALU.mult)
                    stm_tiles[c] = stm[:, :128]
                    stm_tiles[nxt] = stm[:, 128:256]
                else:
                    nc.tensor.matmul(stp[:, :128], lhsT=tchunk(ktf, c),
                                     rhs=tchunk(qtf, c),
                                     start=True, stop=True)
                    nc.vector.scalar_tensor_tensor(
                        stm[:, :128], stp[:, :128], 1.0, masks[:, 0:128],
                        ALU.bypass, ALU.mult)
                    stm_tiles[c] = stm[:, :128]

            for c in range(NCH):
                if c not in stm_tiles:
                    emit_scores(c)
                slot = num_slot(c)
                first = c in seg_start
                bnd = bnd_of.get(c)
                nc.tensor.matmul(slot, lhsT=stm_tiles[c], rhs=vchunk(c),
                                 start=True,
                                 stop=first and bnd is None)
                if not first:
                    nc.tensor.matmul(slot, lhsT=tchunk(qtf, c),
                                     rhs=sslot(c), start=False, stop=True)
                elif bnd is not None:
                    j, a_prev = bnd
                    hb = D * (a_prev >= NCH // 2)
                    nc.tensor.matmul(slot[0:1, :],
                                     lhsT=qxf[hb:hb + D, j:j + 1],
                                     rhs=ps3[hb:hb + D,
                                             j * M1:(j + 1) * M1],
                                     start=False, stop=True,
                                     skip_group_check=True)

                if c % 7 == 6 or c == NCH - 1:
                    g = c // 7
                    n = c - 7 * g + 1
                    dv = pn[:, g, D:D + M1 * (n - 1) + 1:M1]
                    sel = slice(7 * g, 7 * g + n)
                    nc.vector.tensor_scalar_max(dmax[:, sel], dv, EPS)
                    nc.vector.reciprocal(rall[:, sel], dmax[:, sel])
                    for i in range(7 * g, c + 1):
                        dst = outall[:, i * D:(i + 1) * D]
                        if i % 2 == 0:
                            nc.vector.tensor_scalar_mul(
                                dst, num_slot(i)[:, :D],
                                rall[:, i:i + 1])
                        else:
                            nc.scalar.activation(
                                dst, num_slot(i)[:, :D], AF.Copy,
                                scale=rall[:, i:i + 1])
                    # stream finished 8-chunk quarters out
                    done = ((c + 1) // 8) * 8 if c < NCH - 1 else NCH
                    lo, hi = shipped[0] * D, done * D
                    if hi > lo:
                        eng = nc.sync if g % 2 == 0 else nc.scalar
                        eng.dma_start(out=d_out[:, lo:hi],
                                      in_=outall[:, lo:hi])
                        shipped[0] = done
    return nc


# ------------------------------------------------------------------ driver
def _build_masks_fast():
    tri_a = np.triu(np.ones((C, C), np.float32))
    masks = np.zeros((C, 512), np.float32)
    masks[:, 0:128] = tri_a
    masks[:, 256:384] = tri_a
    return masks.astype(np.float16)


def kernel(**inputs):
    q = np.ascontiguousarray(np.asarray(inputs["q"]), dtype=np.float32)
    k = np.ascontiguousarray(np.asarray(inputs["k"]), dtype=np.float32)
    v = np.ascontiguousarray(np.asarray(inputs["v"]), dtype=np.float32)
    seqlens = np.asarray(inputs["seqlens"])
    assert q.shape == (T, H, D), q.shape

    segs = fast_plan(seqlens, WIN)
    if segs is not None:
        nc = build_bass_fast(segs)
        patched = split_waits(nc.to_json_bytes())
        nc.to_json_bytes = lambda: patched
        masks = _build_masks_fast()
        in_maps = []
        for h in range(H):
            in1, k2, q2, qx = pack_head_fast(q[:, h], k[:, h], v[:, h],
                                             segs)
            in_maps.append(dict(in1=in1, k2=k2, q2=q2, qx=qx, masks=masks))
        res = run_bass_kernel_spmd(nc, in_maps, core_ids=list(range(H)),
                                   trace=TRACE)
        if TRACE:
            kernel.last_result = res
        out = np.empty((T, H, D), np.float32)
        for h in range(H):
            out[:, h, :] = unpack_out_fast(res.results[h]["out"])
        return out

    plan = host_plan(seqlens)
    masks, pmask, negalpha, negbeta = build_aux(plan)
    nc = build_bass(plan)
    patched = split_waits(nc.to_json_bytes())
    nc.to_json_bytes = lambda: patched

    in_maps = []
    for h in range(H):
        qtp, ktp, kn, vaug = pack_head(q[:, h], k[:, h], v[:, h])
        im = dict(qtp=qtp, ktp=ktp, kn=kn, vaug=vaug,
                  masks=masks.astype(np.float16),
                  pmask=pmask, negalpha=negalpha, negbeta=negbeta)
        in_maps.append(im)

    res = run_bass_kernel_spmd(nc, in_maps, core_ids=list(range(H)),
                               trace=TRACE)
    if TRACE:
        kernel.last_result = res
    out = np.empty((T, H, D), np.float32)
    for h in range(H):
        out[:, h, :] = res.results[h]["out"]
    return out


# revision 10
# speedup vs baseline: 1.7639x; 1.3837x over previous
"""Trainium2 Bass kernel for segment-packed sliding-window linear attention
(ELU+1 feature map), sharded one head per NeuronCore (8 heads / 8 cores).

Math (per head, per position t):
    qf = elu(q*0.125)+1, kf = elu(k)+1, b(t) = max(seg_start(t), t-1024)
    out[t] = qf_t @ (KV[t]-KVpad[b(t)]) / max(qf_t @ (K[t]-Kpad[b(t)]), eps)
with KV/K *global* causal cumsums of kf (outer) vaug.  Chunked at C=128:
  A  = (Qf Kf_i^T (*) tri<=) Vaug_i + Qf @ S[i]
  B  = active:  (Qf Kf_{i-8}^T (*) tri<) Vaug_{i-8} + Qf @ S[i-8]
       else:    Qf @ P[seg_id]          (prefix matrix per boundary)
  num|den = A - B    (den rides along as Vaug's 65th "ones" column)
All seqlens-dependent control (chunk classification, blend vectors, prefix
masks) is computed host-side and baked into the traced program / tiny aux
inputs.  dens are extracted with two strided batched PSUM reads per pair of
banks.
"""

import numpy as np

import concourse.bass as bass
import concourse.mybir as mybir
import concourse.tile as tile
from concourse.bass_utils import run_bass_kernel_spmd

T, H, D = 4096, 8, 64
C = 128                 # chunk length (partition dim)
NCH = T // C            # 32 chunks
WIN = 1024
WCH = WIN // C          # window = 8 chunks back
M1 = D + 1              # V augmented with ones column -> den for free
SCALE = 0.125
EPS = 1e-6
F32 = mybir.dt.float32
F16 = mybir.dt.float16

TRACE = False           # test harness can flip for NTFF profiling
ALU = mybir.AluOpType
AF = mybir.ActivationFunctionType


# ----------------------------------------------------------------- host plan
def host_plan(seqlens):
    s = np.asarray(seqlens).astype(np.int64)
    assert s.shape[0] >= 2
    pos = np.arange(T)
    seg_id = np.searchsorted(s[1:], pos, side="left")       # [T]
    seg_start = s[seg_id]
    active = seg_start < pos - WIN
    nb = s.shape[0]

    chunks = []
    for i in range(NCH):
        sl = slice(i * C, (i + 1) * C)
        act = active[sl]
        sids = np.unique(seg_id[sl][~act]) if (~act).any() else np.array([], np.int64)
        if act.all():
            chunks.append(dict(kind="W"))
        elif not act.any() and len(sids) == 1:
            chunks.append(dict(kind="S", sid=int(sids[0])))
        else:
            groups = [(int(sid),
                       ((~act) & (seg_id[sl] == sid)).astype(np.float32))
                      for sid in sids]
            chunks.append(dict(kind="G", alpha=act.astype(np.float32),
                               groups=groups))
    bneed = []
    for kc in range(NCH):
        qc = kc + WCH
        if qc >= NCH:
            bneed.append(None)
        else:
            ch = chunks[qc]
            if ch["kind"] == "W":
                bneed.append("neg")
            elif ch["kind"] == "G" and ch["alpha"].any():
                bneed.append("pos")
            else:
                bneed.append(None)
    bnds = []
    for j in range(nb):
        bj = int(np.clip(s[j], 0, T))
        bnds.append((bj // C, bj % C))
    return dict(chunks=chunks, bneed=bneed, bnds=bnds, nb=nb)


def build_aux(plan):
    tri_a = np.triu(np.ones((C, C), np.float32))            # [sl, tl] sl<=tl
    tri_s = np.triu(np.ones((C, C), np.float32), k=1)       # sl<tl
    masks = np.zeros((C, 512), np.float32)
    masks[:, 0:128] = tri_a
    masks[:, 128:256] = -tri_s
    masks[:, 256:384] = tri_a
    masks[:, 384:512] = tri_s

    nb = plan["nb"]
    pmask = np.zeros((C, nb), np.float32)
    for j, (cb, rb) in enumerate(plan["bnds"]):
        pmask[:, j] = (np.arange(C) < rb).astype(np.float32)

    negalpha = np.zeros((C, NCH), np.float32)
    negbeta = np.zeros((C, NCH * nb), np.float32)
    for i, ch in enumerate(plan["chunks"]):
        if ch["kind"] == "G":
            negalpha[:, i] = -ch["alpha"]
            for sid, beta in ch["groups"]:
                negbeta[:, i * nb + sid] = -beta
    return masks, pmask, negalpha, negbeta


def pack_head(q, k, v):
    """q,k,v: [T, D] fp32 one head -> device layouts."""
    qtp = q.T                            # [64, 4096]
    ktp = k.T
    kn = k.reshape(NCH, C, D).transpose(1, 0, 2).reshape(C, NCH * D)
    va = np.concatenate([v.reshape(NCH, C, D),
                         np.ones((NCH, C, 1), np.float32)], axis=2)
    vaug = va.transpose(1, 0, 2).reshape(C, NCH * M1).astype(np.float16)
    return (np.ascontiguousarray(qtp), np.ascontiguousarray(ktp),
            np.ascontiguousarray(kn), np.ascontiguousarray(vaug))


# ------------------------------------------------------------- bass program
def build_bass(plan):
    nb = plan["nb"]
    nc = bass.Bass()
    d_qtp = nc.dram_tensor("qtp", [D, T], F32, kind="ExternalInput")
    d_ktp = nc.dram_tensor("ktp", [D, T], F32, kind="ExternalInput")
    d_kn = nc.dram_tensor("kn", [C, NCH * D], F32, kind="ExternalInput")
    d_vaug = nc.dram_tensor("vaug", [C, NCH * M1], F16,
                            kind="ExternalInput")
    d_masks = nc.dram_tensor("masks", [C, 512], F16, kind="ExternalInput")
    d_pmask = nc.dram_tensor("pmask", [C, nb], F32, kind="ExternalInput")
    d_nalpha = nc.dram_tensor("negalpha", [C, NCH], F32, kind="ExternalInput")
    d_nbeta = nc.dram_tensor("negbeta", [C, NCH * nb], F32,
                             kind="ExternalInput")
    d_out = nc.dram_tensor("out", [T, D], F32, kind="ExternalOutput")

    def tchunk(t, j):
        """[64,128] slice of a transposed [64, T] tensor, chunk j."""
        return t[:, C * j:C * (j + 1)]

    def kchunk(t, c):
        return t[:, c * D:(c + 1) * D]

    def vchunk(t, c):
        return t[:, c * M1:(c + 1) * M1]

    def sslot(t, c):
        return t[:, c * M1:(c + 1) * M1]

    with tile.TileContext(nc) as tc:
        with (
            tc.tile_pool(name="persist", bufs=1) as pp,
            tc.tile_pool(name="stm", bufs=10) as stm_pool,
            tc.tile_pool(name="outp", bufs=8) as out_pool,
            tc.tile_pool(name="pst", bufs=2, space="PSUM") as pst,
            tc.tile_pool(name="pbig", bufs=1, space="PSUM") as pbig,
            tc.tile_pool(name="tmps", bufs=4) as tmp_pool,
        ):
            qtp = pp.tile([D, T], F32)
            ktp = pp.tile([D, T], F32)
            kn = pp.tile([C, NCH * D], F32)
            vaug = pp.tile([C, NCH * M1], F16)
            masks = pp.tile([C, 512], F16)
            pmaskt = pp.tile([C, nb], F32)
            nalpha = pp.tile([C, NCH], F32)
            nbeta = pp.tile([C, NCH * nb], F32)
            sall = pp.tile([D, (NCH + 1) * M1], F16)
            call = pp.tile([D, NCH * M1], F16)
            pall = pp.tile([D, nb * M1], F16)
            rall = pp.tile([C, 42], F32)
            dmax = pp.tile([C, 42], F32)
            e_q = pp.tile([D, T], F16)
            e_k = pp.tile([D, T], F16)
            e_kn = pp.tile([C, NCH * D], F16)
            r_q = pp.tile([D, T], F16)
            r_k = pp.tile([D, T], F16)
            r_kn = pp.tile([C, NCH * D], F16)

            # num slots: one 5-bank PSUM tensor, slot i at 512*(i//7)+65*(i%7)
            pnum = pbig.tile([C, 2560], F32)
            st_ps = pbig.tile([D, M1], F32)   # pass-1 running state

            def num_slot(i):
                off = 512 * (i // 7) + M1 * (i % 7)
                return pnum[:, off:off + M1]

            dma = nc.default_dma_engine
            # kn first: pass-1 and the feat pipeline consume it earliest
            nc.scalar.dma_start(out=kn, in_=d_kn[:, :])
            nc.sync.dma_start(out=qtp, in_=d_qtp[:, :])
            nc.scalar.dma_start(out=ktp, in_=d_ktp[:, :])
            nc.sync.dma_start(out=vaug, in_=d_vaug[:, :])
            nc.scalar.dma_start(out=masks, in_=d_masks[:, :])
            nc.sync.dma_start(out=pmaskt, in_=d_pmask[:, :])
            nc.sync.dma_start(out=nalpha, in_=d_nalpha[:, :])
            nc.sync.dma_start(out=nbeta, in_=d_nbeta[:, :])

            # ---- features: feat(x) = min(exp(s*x),1) + max(s*x,0)
            # kn first (pass-1 consumes it); exp on ACT, relu + fused
            # min/add combine on DVE
            nc.scalar.activation(e_kn, kn, AF.Exp, scale=1.0)
            nc.vector.tensor_scalar(r_kn, kn, 0.0, None, ALU.max)
            nc.vector.scalar_tensor_tensor(e_kn, e_kn, 1.0, r_kn,
                                           ALU.min, ALU.add)
            nc.scalar.activation(e_k, ktp, AF.Exp, scale=1.0)
            nc.vector.tensor_scalar(r_k, ktp, 0.0, None, ALU.max)
            nc.vector.scalar_tensor_tensor(e_k, e_k, 1.0, r_k,
                                           ALU.min, ALU.add)
            nc.scalar.activation(e_q, qtp, AF.Exp, scale=SCALE)
            nc.vector.tensor_scalar(r_q, qtp, 0.0, SCALE, ALU.max, ALU.mult)
            nc.vector.scalar_tensor_tensor(e_q, e_q, 1.0, r_q,
                                           ALU.min, ALU.add)
            qtf, ktf, kf = e_q, e_k, e_kn

            # ---- pass 1: chunk states, running in PSUM, snapshots to SBUF
            nc.vector.memset(sall[:, 0:M1], 0.0)
            for c in range(NCH):
                nc.tensor.matmul(st_ps, lhsT=kchunk(kf, c),
                                 rhs=vchunk(vaug, c),
                                 start=(c == 0), stop=(c == NCH - 1))
                nc.scalar.copy(sslot(sall, c + 1), st_ps)

            # ---- boundary prefix matrices P[j] = cumsum over [0, s_j)
            for j, (cb, rb) in enumerate(plan["bnds"]):
                dst = sslot(pall, j)
                if cb >= NCH:
                    nc.vector.tensor_copy(dst, sslot(sall, NCH))
                elif rb == 0:
                    nc.vector.tensor_copy(dst, sslot(sall, cb))
                else:
                    km = tmp_pool.tile([C, D], F16, tag="km",
                                       name=f"km{j}")
                    nc.vector.tensor_scalar_mul(km, kchunk(kf, cb),
                                                pmaskt[:, j:j + 1])
                    pps = pst.tile([D, M1], F32, tag="st", name=f"pps{j}")
                    nc.tensor.matmul(pps, lhsT=km,
                                     rhs=vchunk(vaug, cb),
                                     start=True, stop=True)
                    nc.vector.scalar_tensor_tensor(dst, pps, 0.0,
                                                   sslot(sall, cb),
                                                   ALU.add, ALU.add)

            # ---- C matrices: W runs batched, S chunks individual
            i = 0
            while i < NCH:
                if plan["chunks"][i]["kind"] == "W":
                    j = i
                    while j < NCH and plan["chunks"][j]["kind"] == "W":
                        j += 1
                    for i0 in range(i, j, 4):
                        n = (min(i0 + 4, j) - i0) * M1
                        nc.vector.scalar_tensor_tensor(
                            call[:, i0 * M1:i0 * M1 + n],
                            sall[:, i0 * M1:i0 * M1 + n], -1.0,
                            sall[:, (i0 - WCH) * M1:(i0 - WCH) * M1 + n],
                            ALU.bypass, ALU.subtract)
                    i = j
                else:
                    i += 1
            for i, ch in enumerate(plan["chunks"]):
                if ch["kind"] == "S":
                    nc.vector.scalar_tensor_tensor(
                        sslot(call, i), sslot(sall, i), -1.0,
                        sslot(pall, ch["sid"]),
                        ALU.bypass, ALU.subtract)

            # ---- pass 2, interleaved: scores for kc=i, then accumulate qc=i
            SV_DT = F16
            vsrc = vaug
            qtf_g = qtf.rearrange("p (g c) -> p g c", c=128)

            def sv_cast(ap):
                return ap

            stm_tiles = {}
            pn = pnum.rearrange("p (b s) -> p b s", s=512)
            masks_g = masks.rearrange("p (x c) -> p x c", c=128)

            def emit_scores(i):
                # scores for key chunk kc=i; two consecutive narrow chunks
                # share one PSUM tile + one mask op (halves DVE op count)
                kc = i
                bm = plan["bneed"][kc]
                wide = bm is not None
                nxt = kc + 1
                pair = (not wide and nxt < NCH
                        and plan["bneed"][nxt] is None)
                stp = pst.tile([C, 256], F32, tag="st", name=f"stp{kc}")
                stm = stm_pool.tile([C, 256], SV_DT, tag="stm",
                                    name=f"stm{kc}")
                if wide:
                    rhs = qtf_g[:, kc:kc + WCH + 1:WCH, :]
                    nc.tensor.matmul(stp, lhsT=tchunk(ktf, kc),
                                     rhs=rhs, start=True, stop=True)
                    moff = 256 if bm == "pos" else 0
                    nc.vector.scalar_tensor_tensor(
                        stm, stp, 1.0, masks[:, moff:moff + 256],
                        ALU.bypass, ALU.mult)
                    stm_tiles[kc] = (stm[:, :128], stm[:, 128:256])
                elif pair:
                    for x, c in enumerate((kc, nxt)):
                        nc.tensor.matmul(stp[:, 128 * x:128 * (x + 1)],
                                         lhsT=tchunk(ktf, c),
                                         rhs=tchunk(qtf, c),
                                         start=True, stop=True)
                    nc.vector.scalar_tensor_tensor(
                        stm.rearrange("p (x c) -> p x c", c=128),
                        stp.rearrange("p (x c) -> p x c", c=128), 1.0,
                        masks_g[:, 0:3:2, :], ALU.bypass, ALU.mult)
                    stm_tiles[kc] = (stm[:, :128], None)
                    stm_tiles[nxt] = (stm[:, 128:256], None)
                else:
                    nc.tensor.matmul(stp[:, :128], lhsT=tchunk(ktf, kc),
                                     rhs=tchunk(qtf, kc),
                                     start=True, stop=True)
                    nc.vector.scalar_tensor_tensor(
                        stm[:, :128], stp[:, :128], 1.0, masks[:, 0:128],
                        ALU.bypass, ALU.mult)
                    stm_tiles[kc] = (stm[:, :128], None)

            for i in range(NCH):
                if i not in stm_tiles:
                    emit_scores(i)

                # accumulate num for query chunk qc=i
                ch = plan["chunks"][i]
                slot = num_slot(i)
                kind = ch["kind"]
                nc.tensor.matmul(slot, lhsT=sv_cast(stm_tiles[i][0]),
                                 rhs=sv_cast(vchunk(vsrc, i)),
                                 start=True, stop=False)
                if kind == "W":
                    nc.tensor.matmul(slot,
                                     lhsT=sv_cast(stm_tiles[i - WCH][1]),
                                     rhs=sv_cast(vchunk(vsrc, i - WCH)),
                                     start=False, stop=False)
                    nc.tensor.matmul(slot, lhsT=tchunk(qtf, i),
                                     rhs=sslot(call, i),
                                     start=False, stop=True)
                elif kind == "S":
                    nc.tensor.matmul(slot, lhsT=tchunk(qtf, i),
                                     rhs=sslot(call, i),
                                     start=False, stop=True)
                else:  # G
                    nc.tensor.matmul(slot, lhsT=tchunk(qtf, i),
                                     rhs=sslot(sall, i),
                                     start=False, stop=True)
                    terms = []
                    if ch["alpha"].any():
                        bw = pst.tile([C, M1], F32, tag="st", name=f"bw{i}")
                        nc.tensor.matmul(
                            bw, lhsT=sv_cast(stm_tiles[i - WCH][1]),
                            rhs=sv_cast(vchunk(vsrc, i - WCH)),
                            start=True, stop=False)
                        nc.tensor.matmul(bw, lhsT=tchunk(qtf, i),
                                         rhs=sslot(sall, i - WCH),
                                         start=False, stop=True)
                        terms.append((bw, nalpha[:, i:i + 1]))
                    for sid, _ in ch["groups"]:
                        gp = pst.tile([C, M1], F32, tag="st",
                                      name=f"gp{i}_{sid}")
                        nc.tensor.matmul(gp, lhsT=tchunk(qtf, i),
                                         rhs=sslot(pall, sid),
                                         start=True, stop=True)
                        terms.append((gp, nbeta[:, i * nb + sid:
                                                i * nb + sid + 1]))
                    # fold: slot = main + sum(term * negscale).
                    # DVE reads at most one PSUM operand per op, so move the
                    # main accumulator to SBUF first, then chain terms.
                    acc = tmp_pool.tile([C, M1], F32, tag="gt",
                                        name=f"gacc{i}")
                    nc.scalar.copy(acc, slot)
                    for t_idx, (tps, sc) in enumerate(terms):
                        last = t_idx == len(terms) - 1
                        dst = slot if last else tmp_pool.tile(
                            [C, M1], F32, tag="gt", name=f"gt{i}_{t_idx}")
                        nc.vector.scalar_tensor_tensor(
                            dst, tps, sc, acc, ALU.mult, ALU.add)
                        acc = dst

                # dens for a completed PSUM bank, emitted inline so they
                # run ahead of the remaining mask ops in DVE's queue
                if i % 7 == 6 or i == NCH - 1:
                    g = i // 7
                    dv = pn[:, g, D:D + 65 * 6 + 1:65]
                    sel = slice(7 * g, 7 * g + 7)
                    nc.vector.tensor_scalar_max(dmax[:, sel], dv, EPS)
                    nc.vector.reciprocal(rall[:, sel], dmax[:, sel])

            # ---- scale + store
            for i in range(NCH):
                ob = out_pool.tile([C, D], F32, tag="ob", name=f"ob{i}")
                nc.scalar.activation(ob, num_slot(i)[:, :D], AF.Copy,
                                     scale=rall[:, i:i + 1])
                eng = nc.sync if i % 2 == 0 else nc.scalar
                eng.dma_start(out=d_out[i * C:(i + 1) * C, :], in_=ob)
    return nc


def split_waits(bir: bytes) -> bytes:
    """Walrus codegen caps sync waits at 1 per instruction (2 for
    EventSemaphore); Tile sometimes attaches more.  Hoist the excess into
    preceding same-engine NoOps (engines are in-order, so semantics hold)."""
    import json
    m = json.loads(bir)
    for f in m["functions"]:
        for bb in f["blocks"]:
            out = []
            for ins in bb["instructions"]:
                si = ins.get("sync_info")
                ow = (si or {}).get("on_wait") or []
                cap = 2 if ins.get("opcode") == "EventSemaphore" else 1
                eng = ins.get("engine")
                if eng and len(ow) > cap:
                    keep = ow[-cap:]
                    for j, w in enumerate(ow[:-cap]):
                        out.append({"name": f'{ins["name"]}_sw{j}',
                                    "opcode": "NoOp", "engine": eng,
                                    "ins": [], "outs": [],
                                    "sync_info": {"on_wait": [w],
                                                  "on_update": []}})
                    ins = dict(ins)
                    ins["sync_info"] = {
                        "on_wait": keep,
                        "on_update": (si or {}).get("on_update") or []}
                out.append(ins)
            bb["instructions"] = out
    return json.dumps(m).encode()


# ===================================================== fast path
TH = T // 2             # 2048, packed free dim


def fast_plan(seqlens, win):
    """Return list of segment chunk-ranges if the fast path applies, else None."""
    s = np.asarray(seqlens).astype(np.int64)
    s = np.clip(s, 0, T)
    b = np.unique(np.concatenate([[0], s, [T]]))
    if b[0] != 0 or b[-1] != T:
        return None
    if (b % C).any():
        return None
    segs = []
    for a, e in zip(b[:-1], b[1:]):
        if e - a > win:          # sliding window would activate
            return None
        ca, ce = int(a) // C, int(e) // C
        if ca < NCH // 2 < ce:
            return None          # segment straddles the packing boundary
        segs.append((ca, ce))
    return segs


def pack_head_fast(q, k, v, segs):
    """q,k,v: [T, D] fp32 -> (in1, k2, q2, qx) device layouts (f16)."""
    kn = k.reshape(NCH, C, D).transpose(1, 0, 2).reshape(C, NCH * D)
    va = np.concatenate([v.reshape(NCH, C, D),
                         np.ones((NCH, C, 1), np.float32)], axis=2)
    vaug = va.transpose(1, 0, 2).reshape(C, NCH * M1)
    in1 = np.concatenate([kn, vaug], axis=1).astype(np.float16)
    k2 = k.T.reshape(D, 2, TH).transpose(1, 0, 2).reshape(2 * D, TH)
    q2 = q.T.reshape(D, 2, TH).transpose(1, 0, 2).reshape(2 * D, TH)
    # boundary-query columns: query at t = e*C attends over the whole
    # previous segment (searchsorted side='left' semantics); place its raw
    # q column at the previous segment's partition half.
    bnds = [(a, e) for a, e in segs if e < NCH]
    qx = np.zeros((C, max(1, len(bnds))), np.float32)
    for j, (a, e) in enumerate(bnds):
        hb = D if a >= NCH // 2 else 0
        qx[hb:hb + D, j] = q[e * C, :]
    return (np.ascontiguousarray(in1),
            np.ascontiguousarray(k2.astype(np.float16)),
            np.ascontiguousarray(q2.astype(np.float16)),
            np.ascontiguousarray(qx.astype(np.float16)))


def unpack_out_fast(res):
    """[128, NCH*64] f16 chunk-major -> [T, D] f32"""
    o = np.asarray(res, np.float32).reshape(C, NCH, D)
    return o.transpose(1, 0, 2).reshape(T, D)


def build_bass_fast(segs):
    bnds = [(a, e) for a, e in segs if e < NCH]
    NB = max(1, len(bnds))
    canonical = segs == [(0, 8), (8, 16), (16, 24), (24, 32)]
    nc = bass.Bass()
    d_in1 = nc.dram_tensor("in1", [C, NCH * D + NCH * M1], F16,
                           kind="ExternalInput")
    d_k2 = nc.dram_tensor("k2", [C, TH], F16, kind="ExternalInput")
    d_q2 = nc.dram_tensor("q2", [C, TH], F16, kind="ExternalInput")
    d_qx = nc.dram_tensor("qx", [C, NB], F16, kind="ExternalInput")
    d_masks = nc.dram_tensor("masks", [C, 512], F16, kind="ExternalInput")
    d_out = nc.dram_tensor("out", [C, NCH * D], F16, kind="ExternalOutput")

    seg_start = set(a for a, e in segs)
    bnd_of = {e: (j, a) for j, (a, e) in enumerate(bnds)}
    seg_idx = {}
    for si, (a, e) in enumerate(segs):
        for c in range(a, e):
            seg_idx[c] = (si, a)

    HC = NCH // 2        # chunks per packing half
    HT = TH // 2         # 1024: free-dim half of the packed tensors

    def tchunk(t, j):
        h, r = divmod(j, HC)
        return t[h * D:(h + 1) * D, r * C:(r + 1) * C]

    def kchunk(t, c):
        return t[:, c * D:(c + 1) * D]

    with tile.TileContext(nc) as tc:
        with (
            tc.tile_pool(name="persist", bufs=1) as pp,
            tc.tile_pool(name="stm", bufs=4) as stm_pool,
            tc.tile_pool(name="pst", bufs=2, space="PSUM") as pst,
            tc.tile_pool(name="pnump", bufs=2, space="PSUM") as pnump,
            tc.tile_pool(name="pgbig", bufs=1, space="PSUM") as pgbig,
        ):
            in1 = pp.tile([C, NCH * D + NCH * M1], F16)
            kn = in1[:, :NCH * D]
            vaug = in1[:, NCH * D:]
            k2 = pp.tile([C, TH], F16)
            q2 = pp.tile([C, TH], F16)
            kf = pp.tile([C, NCH * D], F16)
            ktf = pp.tile([C, TH], F16)
            qtf = pp.tile([C, TH], F16)
            rel_kn = pp.tile([C, NCH * D], F16)
            rel_k = pp.tile([C, TH], F16)
            rel_q = pp.tile([C, TH], F16)
            masks = pp.tile([C, 512], F16)
            rall = pp.tile([C, NCH], F32)
            dmax = pp.tile([C, NCH], F32)
            outall = pp.tile([C, NCH * D], F16)
            qx = pp.tile([C, NB], F16)
            qxf = pp.tile([C, NB], F16)
            qxr = pp.tile([C, NB], F16)
            ps3 = pp.tile([C, NB * M1], F16)
            tiny = pp.tile([1, 1], F32)
            # states: canonical = 7 steps x (2 segs x 65) per half;
            # general = one 65-slot per chunk, half-packed
            sall = pp.tile([C, (7 * 2 if canonical else HC) * M1], F16)

            # pass-1 outer products; bank-safe layout: per half, steps 0-6
            # of seg-even in bank 0, seg-odd in bank 1, segment-final G's in
            # bank 2 (PSUM matmul outputs must not cross a 2KB bank)
            G2 = pgbig.tile([C, 1536], F32)

            def g2slot(c):
                h, r = divmod(c, HC)
                if r % 8 < 7:
                    off = 512 * (r // 8) + M1 * (r % 8)
                else:
                    off = 1024 + M1 * (r // 8)
                return G2[h * D:(h + 1) * D, off:off + M1]

            def sstate(c):
                """SBUF state (exclusive prefix) for chunk c."""
                if canonical:
                    s, j = divmod(c, 8)
                    hb = D * (s // 2)
                    off = (j - 1) * 2 * M1 + (s % 2) * M1
                    return sall[hb:hb + D, off:off + M1]
                h, r = divmod(c, HC)
                return sall[h * D:(h + 1) * D, r * M1:(r + 1) * M1]

            # --- warm the ACT exp table before any data arrives
            nc.gpsimd.memset(tiny, 0.0)
            nc.scalar.activation(tiny, tiny, AF.Exp, scale=1.0)

            # --- input DMAs on the scalar HWDGE ring (starts ~3us earlier
            # than the sync ring); outputs use the sync ring
            nc.scalar.dma_start(out=masks, in_=d_masks[:, :])
            nc.scalar.dma_start(out=qx, in_=d_qx[:, :])
            nc.scalar.dma_start(out=in1, in_=d_in1[:, :])
            nc.scalar.dma_start(out=k2[:, 0:HT], in_=d_k2[:, 0:HT])
            nc.scalar.dma_start(out=q2[:, 0:HT], in_=d_q2[:, 0:HT])
            nc.scalar.dma_start(out=k2[:, HT:TH], in_=d_k2[:, HT:TH])
            nc.scalar.dma_start(out=q2[:, HT:TH], in_=d_q2[:, HT:TH])

            # --- features: f(x) = min(exp(s x),1) + max(s x,0)
            # exp on ACT; max/min (4x) + add (2x) on DVE
            def feat_exp(dst, src, scale):
                nc.scalar.activation(dst, src, AF.Exp, scale=scale)

            def feat_rest(dst, src, rel, scale):
                if scale == 1.0:
                    nc.vector.tensor_scalar(rel, src, 0.0, None, ALU.max)
                else:
                    nc.vector.tensor_scalar(rel, src, 0.0, scale,
                                            ALU.max, ALU.mult)
                nc.vector.tensor_scalar_min(dst, dst, 1.0)
                nc.vector.tensor_add(dst, dst, rel)

            feat_exp(kf, kn, 1.0)
            feat_exp(ktf[:, 0:HT], k2[:, 0:HT], 1.0)
            feat_exp(qtf[:, 0:HT], q2[:, 0:HT], SCALE)
            feat_exp(ktf[:, HT:TH], k2[:, HT:TH], 1.0)
            feat_exp(qtf[:, HT:TH], q2[:, HT:TH], SCALE)
            feat_exp(qxf, qx, SCALE)

            feat_rest(kf, kn, rel_kn, 1.0)
            feat_rest(ktf[:, 0:HT], k2[:, 0:HT], rel_k[:, 0:HT], 1.0)
            feat_rest(qtf[:, 0:HT], q2[:, 0:HT], rel_q[:, 0:HT], SCALE)

            # --- pass 1: independent outer products into G2
            used = [c for c in range(NCH - 1)
                    if (c + 1 not in seg_start) or (c + 1 in bnd_of)]
            for c in used:
                nc.tensor.matmul(g2slot(c), lhsT=kchunk(kf, c),
                                 rhs=vchunk_of(vaug, c), start=True,
                                 stop=True)

            # --- prefix scan -> SBUF states
            G2r = G2.rearrange("p (b s) -> p b s", s=512)
            if canonical:
                sv = sall.rearrange("p (s c) -> p s c", c=M1)
                for j in range(1, 8):
                    dst = sv[:, 2 * (j - 1):2 * j, :]
                    gsrc = G2r[:, 0:2, M1 * (j - 1):M1 * j]
                    if j == 1:
                        nc.vector.tensor_copy(dst, gsrc)
                    else:
                        nc.vector.scalar_tensor_tensor(
                            dst, gsrc, 0.0,
                            sv[:, 2 * (j - 2):2 * (j - 1), :],
                            ALU.bypass, ALU.add)
            else:
                for a, e in segs:
                    for c in range(a + 1, e):
                        if c == a + 1:
                            nc.vector.tensor_copy(sstate(c), g2slot(c - 1))
                        else:
                            nc.vector.scalar_tensor_tensor(
                                sstate(c), g2slot(c - 1), 0.0,
                                sstate(c - 1), ALU.bypass, ALU.add)
            # full previous-segment sums for the boundary queries
            for a, e in segs:
                if e in bnd_of:
                    j, _ = bnd_of[e]
                    hb = D * (a >= HC)
                    dst = ps3[hb:hb + D, j * M1:(j + 1) * M1]
                    if e - 1 == a:
                        nc.vector.tensor_copy(dst, g2slot(e - 1))
                    else:
                        nc.vector.scalar_tensor_tensor(
                            dst, g2slot(e - 1), 0.0, sstate(e - 1),
                            ALU.bypass, ALU.add)

            # remaining features (second halves + boundary queries)
            feat_rest(ktf[:, HT:TH], k2[:, HT:TH], rel_k[:, HT:TH], 1.0)
            feat_rest(qtf[:, HT:TH], q2[:, HT:TH], rel_q[:, HT:TH], SCALE)
            nc.vector.tensor_scalar(qxr, qx, 0.0, SCALE,
                                    ALU.max, ALU.mult)
            nc.vector.tensor_scalar_min(qxf, qxf, 1.0)
            nc.vector.tensor_add(qxf, qxf, qxr)

            # --- pass 2
            masks_q = masks.rearrange("p (x c) -> p x c", c=128)
            stm_tiles = {}
            gtiles = {}
            shipped = [0]

            def emit_quad(qd):
                c0 = qd * 4
                ncn = min(4, NCH - c0)
                stp = pst.tile([C, 512], F32, tag="st", name=f"stp{qd}")
                stm = stm_pool.tile([C, 512], F16, tag="stm",
                                    name=f"stm{qd}")
                for x in range(ncn):
                    cc = c0 + x
                    nc.tensor.matmul(stp[:, 128 * x:128 * (x + 1)],
                                     lhsT=tchunk(ktf, cc),
                                     rhs=tchunk(qtf, cc),
                                     start=True, stop=True)
                nn = 128 * ncn
                nc.vector.scalar_tensor_tensor(
                    stm[:, :nn].rearrange("p (x c) -> p x c", c=128),
                    stp[:, :nn].rearrange("p (x c) -> p x c", c=128), 1.0,
                    masks_q[:, 0:1, :].broadcast_to([C, ncn, 128]),
                    ALU.bypass, ALU.mult)
                for x in range(ncn):
                    stm_tiles[c0 + x] = stm[:, 128 * x:128 * (x + 1)]

            emit_quad(0)
            emit_quad(1)

            for c in range(NCH):
                g = c // 7
                if c % 7 == 0:
                    gtiles[g] = pnump.tile([C, 512], F32, tag="num",
                                           name=f"num{g}")
                slot = gtiles[g][:, (c - 7 * g) * M1:(c - 7 * g + 1) * M1]
                first = c in seg_start
                bnd = bnd_of.get(c)
                nc.tensor.matmul(slot, lhsT=stm_tiles[c],
                                 rhs=vchunk_of(vaug, c),
                                 start=True, stop=first and bnd is None)
                if not first:
                    nc.tensor.matmul(slot, lhsT=tchunk(qtf, c),
                                     rhs=sstate(c), start=False, stop=True)
                elif bnd is not None:
                    j, a_prev = bnd
                    hb = D * (a_prev >= HC)
                    nc.tensor.matmul(slot[0:1, :],
                                     lhsT=qxf[hb:hb + D, j:j + 1],
                                     rhs=ps3[hb:hb + D,
                                             j * M1:(j + 1) * M1],
                                     start=False, stop=True,
                                     skip_group_check=True)
                if c % 4 == 3 and (c // 4 + 2) * 4 < NCH:
                    emit_quad(c // 4 + 2)

                if c % 7 == 6 or c == NCH - 1:
                    n = c - 7 * g + 1
                    gt = gtiles[g]
                    dv = gt[:, D:D + M1 * (n - 1) + 1:M1]
                    sel = slice(7 * g, 7 * g + n)
                    nc.vector.tensor_scalar_max(dmax[:, sel], dv, EPS)
                    nc.vector.reciprocal(rall[:, sel], dmax[:, sel])
                    # batched scale: out[p,i,m] = num[p,i*65+m] * rall[p,i]
                    pv = gt[:, 0:455].rearrange("p (i m) -> p i m",
                                                m=M1)[:, 0:n, 0:D]
                    rv = rall[:, sel].rearrange(
                        "p (i u) -> p i u", u=1).broadcast_to([C, n, D])
                    ov = outall[:, 7 * g * D:(7 * g + n) * D].rearrange(
                        "p (i m) -> p i m", m=D)
                    nc.vector.tensor_tensor(ov, pv, rv, ALU.mult)
                    done = ((c + 1) // 8) * 8 if c < NCH - 1 else NCH
                    lo, hi = shipped[0] * D, done * D
                    if hi > lo:
                        nc.sync.dma_start(out=d_out[:, lo:hi],
                                          in_=outall[:, lo:hi])
                        shipped[0] = done
    return nc


def vchunk_of(vaug, c):
    return vaug[:, c * M1:(c + 1) * M1]


# ------------------------------------------------------------------ driver
def _build_masks_fast():
    tri_a = np.triu(np.ones((C, C), np.float32))
    masks = np.zeros((C, 512), np.float32)
    masks[:, 0:128] = tri_a
    masks[:, 256:384] = tri_a
    return masks.astype(np.float16)


def kernel(**inputs):
    q = np.ascontiguousarray(np.asarray(inputs["q"]), dtype=np.float32)
    k = np.ascontiguousarray(np.asarray(inputs["k"]), dtype=np.float32)
    v = np.ascontiguousarray(np.asarray(inputs["v"]), dtype=np.float32)
    seqlens = np.asarray(inputs["seqlens"])
    assert q.shape == (T, H, D), q.shape

    segs = fast_plan(seqlens, WIN)
    if segs is not None:
        nc = build_bass_fast(segs)
        patched = split_waits(nc.to_json_bytes())
        nc.to_json_bytes = lambda: patched
        masks = _build_masks_fast()
        in_maps = []
        for h in range(H):
            in1, k2, q2, qx = pack_head_fast(q[:, h], k[:, h], v[:, h],
                                             segs)
            in_maps.append(dict(in1=in1, k2=k2, q2=q2, qx=qx, masks=masks))
        res = run_bass_kernel_spmd(nc, in_maps, core_ids=list(range(H)),
                                   trace=TRACE)
        if TRACE:
            kernel.last_result = res
        out = np.empty((T, H, D), np.float32)
        for h in range(H):
            out[:, h, :] = unpack_out_fast(res.results[h]["out"])
        return out

    plan = host_plan(seqlens)
    masks, pmask, negalpha, negbeta = build_aux(plan)
    nc = build_bass(plan)
    patched = split_waits(nc.to_json_bytes())
    nc.to_json_bytes = lambda: patched

    in_maps = []
    for h in range(H):
        qtp, ktp, kn, vaug = pack_head(q[:, h], k[:, h], v[:, h])
        im = dict(qtp=qtp, ktp=ktp, kn=kn, vaug=vaug,
                  masks=masks.astype(np.float16),
                  pmask=pmask, negalpha=negalpha, negbeta=negbeta)
        in_maps.append(im)

    res = run_bass_kernel_spmd(nc, in_maps, core_ids=list(range(H)),
                               trace=TRACE)
    if TRACE:
        kernel.last_result = res
    out = np.empty((T, H, D), np.float32)
    for h in range(H):
        out[:, h, :] = res.results[h]["out"]
    return out


# revision 17
# speedup vs baseline: 1.8449x; 1.0459x over previous
"""Trainium2 Bass kernel for segment-packed sliding-window linear attention
(ELU+1 feature map), sharded one head per NeuronCore (8 heads / 8 cores).

Math (per head, per position t):
    qf = elu(q*0.125)+1, kf = elu(k)+1, b(t) = max(seg_start(t), t-1024)
    out[t] = qf_t @ (KV[t]-KVpad[b(t)]) / max(qf_t @ (K[t]-Kpad[b(t)]), eps)
with KV/K *global* causal cumsums of kf (outer) vaug.  Chunked at C=128:
  A  = (Qf Kf_i^T (*) tri<=) Vaug_i + Qf @ S[i]
  B  = active:  (Qf Kf_{i-8}^T (*) tri<) Vaug_{i-8} + Qf @ S[i-8]
       else:    Qf @ P[seg_id]          (prefix matrix per boundary)
  num|den = A - B    (den rides along as Vaug's 65th "ones" column)
All seqlens-dependent control (chunk classification, blend vectors, prefix
masks) is computed host-side and baked into the traced program / tiny aux
inputs.  dens are extracted with two strided batched PSUM reads per pair of
banks.
"""

import numpy as np

import concourse.bass as bass
import concourse.mybir as mybir
import concourse.tile as tile
from concourse.bass_utils import run_bass_kernel_spmd

T, H, D = 4096, 8, 64
C = 128                 # chunk length (partition dim)
NCH = T // C            # 32 chunks
WIN = 1024
WCH = WIN // C          # window = 8 chunks back
M1 = D + 1              # V augmented with ones column -> den for free
SCALE = 0.125
EPS = 1e-6
F32 = mybir.dt.float32
F16 = mybir.dt.float16

TRACE = False           # test harness can flip for NTFF profiling
ALU = mybir.AluOpType
AF = mybir.ActivationFunctionType


# ----------------------------------------------------------------- host plan
def host_plan(seqlens):
    s = np.asarray(seqlens).astype(np.int64)
    assert s.shape[0] >= 2
    pos = np.arange(T)
    seg_id = np.searchsorted(s[1:], pos, side="left")       # [T]
    seg_start = s[seg_id]
    active = seg_start < pos - WIN
    nb = s.shape[0]

    chunks = []
    for i in range(NCH):
        sl = slice(i * C, (i + 1) * C)
        act = active[sl]
        sids = np.unique(seg_id[sl][~act]) if (~act).any() else np.array([], np.int64)
        if act.all():
            chunks.append(dict(kind="W"))
        elif not act.any() and len(sids) == 1:
            chunks.append(dict(kind="S", sid=int(sids[0])))
        else:
            groups = [(int(sid),
                       ((~act) & (seg_id[sl] == sid)).astype(np.float32))
                      for sid in sids]
            chunks.append(dict(kind="G", alpha=act.astype(np.float32),
                               groups=groups))
    bneed = []
    for kc in range(NCH):
        qc = kc + WCH
        if qc >= NCH:
            bneed.append(None)
        else:
            ch = chunks[qc]
            if ch["kind"] == "W":
                bneed.append("neg")
            elif ch["kind"] == "G" and ch["alpha"].any():
                bneed.append("pos")
            else:
                bneed.append(None)
    bnds = []
    for j in range(nb):
        bj = int(np.clip(s[j], 0, T))
        bnds.append((bj // C, bj % C))
    return dict(chunks=chunks, bneed=bneed, bnds=bnds, nb=nb)


def build_aux(plan):
    tri_a = np.triu(np.ones((C, C), np.float32))            # [sl, tl] sl<=tl
    tri_s = np.triu(np.ones((C, C), np.float32), k=1)       # sl<tl
    masks = np.zeros((C, 512), np.float32)
    masks[:, 0:128] = tri_a
    masks[:, 128:256] = -tri_s
    masks[:, 256:384] = tri_a
    masks[:, 384:512] = tri_s

    nb = plan["nb"]
    pmask = np.zeros((C, nb), np.float32)
    for j, (cb, rb) in enumerate(plan["bnds"]):
        pmask[:, j] = (np.arange(C) < rb).astype(np.float32)

    negalpha = np.zeros((C, NCH), np.float32)
    negbeta = np.zeros((C, NCH * nb), np.float32)
    for i, ch in enumerate(plan["chunks"]):
        if ch["kind"] == "G":
            negalpha[:, i] = -ch["alpha"]
            for sid, beta in ch["groups"]:
                negbeta[:, i * nb + sid] = -beta
    return masks, pmask, negalpha, negbeta


def pack_head(q, k, v):
    """q,k,v: [T, D] fp32 one head -> device layouts."""
    qtp = q.T                            # [64, 4096]
    ktp = k.T
    kn = k.reshape(NCH, C, D).transpose(1, 0, 2).reshape(C, NCH * D)
    va = np.concatenate([v.reshape(NCH, C, D),
                         np.ones((NCH, C, 1), np.float32)], axis=2)
    vaug = va.transpose(1, 0, 2).reshape(C, NCH * M1).astype(np.float16)
    return (np.ascontiguousarray(qtp), np.ascontiguousarray(ktp),
            np.ascontiguousarray(kn), np.ascontiguousarray(vaug))


# ------------------------------------------------------------- bass program
def build_bass(plan):
    nb = plan["nb"]
    nc = bass.Bass()
    d_qtp = nc.dram_tensor("qtp", [D, T], F32, kind="ExternalInput")
    d_ktp = nc.dram_tensor("ktp", [D, T], F32, kind="ExternalInput")
    d_kn = nc.dram_tensor("kn", [C, NCH * D], F32, kind="ExternalInput")
    d_vaug = nc.dram_tensor("vaug", [C, NCH * M1], F16,
                            kind="ExternalInput")
    d_masks = nc.dram_tensor("masks", [C, 512], F16, kind="ExternalInput")
    d_pmask = nc.dram_tensor("pmask", [C, nb], F32, kind="ExternalInput")
    d_nalpha = nc.dram_tensor("negalpha", [C, NCH], F32, kind="ExternalInput")
    d_nbeta = nc.dram_tensor("negbeta", [C, NCH * nb], F32,
                             kind="ExternalInput")
    d_out = nc.dram_tensor("out", [T, D], F32, kind="ExternalOutput")

    def tchunk(t, j):
        """[64,128] slice of a transposed [64, T] tensor, chunk j."""
        return t[:, C * j:C * (j + 1)]

    def kchunk(t, c):
        return t[:, c * D:(c + 1) * D]

    def vchunk(t, c):
        return t[:, c * M1:(c + 1) * M1]

    def sslot(t, c):
        return t[:, c * M1:(c + 1) * M1]

    with tile.TileContext(nc) as tc:
        with (
            tc.tile_pool(name="persist", bufs=1) as pp,
            tc.tile_pool(name="stm", bufs=10) as stm_pool,
            tc.tile_pool(name="outp", bufs=8) as out_pool,
            tc.tile_pool(name="pst", bufs=2, space="PSUM") as pst,
            tc.tile_pool(name="pbig", bufs=1, space="PSUM") as pbig,
            tc.tile_pool(name="tmps", bufs=4) as tmp_pool,
        ):
            qtp = pp.tile([D, T], F32)
            ktp = pp.tile([D, T], F32)
            kn = pp.tile([C, NCH * D], F32)
            vaug = pp.tile([C, NCH * M1], F16)
            masks = pp.tile([C, 512], F16)
            pmaskt = pp.tile([C, nb], F32)
            nalpha = pp.tile([C, NCH], F32)
            nbeta = pp.tile([C, NCH * nb], F32)
            sall = pp.tile([D, (NCH + 1) * M1], F16)
            call = pp.tile([D, NCH * M1], F16)
            pall = pp.tile([D, nb * M1], F16)
            rall = pp.tile([C, 42], F32)
            dmax = pp.tile([C, 42], F32)
            e_q = pp.tile([D, T], F16)
            e_k = pp.tile([D, T], F16)
            e_kn = pp.tile([C, NCH * D], F16)
            r_q = pp.tile([D, T], F16)
            r_k = pp.tile([D, T], F16)
            r_kn = pp.tile([C, NCH * D], F16)

            # num slots: one 5-bank PSUM tensor, slot i at 512*(i//7)+65*(i%7)
            pnum = pbig.tile([C, 2560], F32)
            st_ps = pbig.tile([D, M1], F32)   # pass-1 running state

            def num_slot(i):
                off = 512 * (i // 7) + M1 * (i % 7)
                return pnum[:, off:off + M1]

            dma = nc.default_dma_engine
            # kn first: pass-1 and the feat pipeline consume it earliest
            nc.scalar.dma_start(out=kn, in_=d_kn[:, :])
            nc.sync.dma_start(out=qtp, in_=d_qtp[:, :])
            nc.scalar.dma_start(out=ktp, in_=d_ktp[:, :])
            nc.sync.dma_start(out=vaug, in_=d_vaug[:, :])
            nc.scalar.dma_start(out=masks, in_=d_masks[:, :])
            nc.sync.dma_start(out=pmaskt, in_=d_pmask[:, :])
            nc.sync.dma_start(out=nalpha, in_=d_nalpha[:, :])
            nc.sync.dma_start(out=nbeta, in_=d_nbeta[:, :])

            # ---- features: feat(x) = min(exp(s*x),1) + max(s*x,0)
            # kn first (pass-1 consumes it); exp on ACT, relu + fused
            # min/add combine on DVE
            nc.scalar.activation(e_kn, kn, AF.Exp, scale=1.0)
            nc.vector.tensor_scalar(r_kn, kn, 0.0, None, ALU.max)
            nc.vector.scalar_tensor_tensor(e_kn, e_kn, 1.0, r_kn,
                                           ALU.min, ALU.add)
            nc.scalar.activation(e_k, ktp, AF.Exp, scale=1.0)
            nc.vector.tensor_scalar(r_k, ktp, 0.0, None, ALU.max)
            nc.vector.scalar_tensor_tensor(e_k, e_k, 1.0, r_k,
                                           ALU.min, ALU.add)
            nc.scalar.activation(e_q, qtp, AF.Exp, scale=SCALE)
            nc.vector.tensor_scalar(r_q, qtp, 0.0, SCALE, ALU.max, ALU.mult)
            nc.vector.scalar_tensor_tensor(e_q, e_q, 1.0, r_q,
                                           ALU.min, ALU.add)
            qtf, ktf, kf = e_q, e_k, e_kn

            # ---- pass 1: chunk states, running in PSUM, snapshots to SBUF
            nc.vector.memset(sall[:, 0:M1], 0.0)
            for c in range(NCH):
                nc.tensor.matmul(st_ps, lhsT=kchunk(kf, c),
                                 rhs=vchunk(vaug, c),
                                 start=(c == 0), stop=(c == NCH - 1))
                nc.scalar.copy(sslot(sall, c + 1), st_ps)

            # ---- boundary prefix matrices P[j] = cumsum over [0, s_j)
            for j, (cb, rb) in enumerate(plan["bnds"]):
                dst = sslot(pall, j)
                if cb >= NCH:
                    nc.vector.tensor_copy(dst, sslot(sall, NCH))
                elif rb == 0:
                    nc.vector.tensor_copy(dst, sslot(sall, cb))
                else:
                    km = tmp_pool.tile([C, D], F16, tag="km",
                                       name=f"km{j}")
                    nc.vector.tensor_scalar_mul(km, kchunk(kf, cb),
                                                pmaskt[:, j:j + 1])
                    pps = pst.tile([D, M1], F32, tag="st", name=f"pps{j}")
                    nc.tensor.matmul(pps, lhsT=km,
                                     rhs=vchunk(vaug, cb),
                                     start=True, stop=True)
                    nc.vector.scalar_tensor_tensor(dst, pps, 0.0,
                                                   sslot(sall, cb),
                                                   ALU.add, ALU.add)

            # ---- C matrices: W runs batched, S chunks individual
            i = 0
            while i < NCH:
                if plan["chunks"][i]["kind"] == "W":
                    j = i
                    while j < NCH and plan["chunks"][j]["kind"] == "W":
                        j += 1
                    for i0 in range(i, j, 4):
                        n = (min(i0 + 4, j) - i0) * M1
                        nc.vector.scalar_tensor_tensor(
                            call[:, i0 * M1:i0 * M1 + n],
                            sall[:, i0 * M1:i0 * M1 + n], -1.0,
                            sall[:, (i0 - WCH) * M1:(i0 - WCH) * M1 + n],
                            ALU.bypass, ALU.subtract)
                    i = j
                else:
                    i += 1
            for i, ch in enumerate(plan["chunks"]):
                if ch["kind"] == "S":
                    nc.vector.scalar_tensor_tensor(
                        sslot(call, i), sslot(sall, i), -1.0,
                        sslot(pall, ch["sid"]),
                        ALU.bypass, ALU.subtract)

            # ---- pass 2, interleaved: scores for kc=i, then accumulate qc=i
            SV_DT = F16
            vsrc = vaug
            qtf_g = qtf.rearrange("p (g c) -> p g c", c=128)

            def sv_cast(ap):
                return ap

            stm_tiles = {}
            pn = pnum.rearrange("p (b s) -> p b s", s=512)
            masks_g = masks.rearrange("p (x c) -> p x c", c=128)

            def emit_scores(i):
                # scores for key chunk kc=i; two consecutive narrow chunks
                # share one PSUM tile + one mask op (halves DVE op count)
                kc = i
                bm = plan["bneed"][kc]
                wide = bm is not None
                nxt = kc + 1
                pair = (not wide and nxt < NCH
                        and plan["bneed"][nxt] is None)
                stp = pst.tile([C, 256], F32, tag="st", name=f"stp{kc}")
                stm = stm_pool.tile([C, 256], SV_DT, tag="stm",
                                    name=f"stm{kc}")
                if wide:
                    rhs = qtf_g[:, kc:kc + WCH + 1:WCH, :]
                    nc.tensor.matmul(stp, lhsT=tchunk(ktf, kc),
                                     rhs=rhs, start=True, stop=True)
                    moff = 256 if bm == "pos" else 0
                    nc.vector.scalar_tensor_tensor(
                        stm, stp, 1.0, masks[:, moff:moff + 256],
                        ALU.bypass, ALU.mult)
                    stm_tiles[kc] = (stm[:, :128], stm[:, 128:256])
                elif pair:
                    for x, c in enumerate((kc, nxt)):
                        nc.tensor.matmul(stp[:, 128 * x:128 * (x + 1)],
                                         lhsT=tchunk(ktf, c),
                                         rhs=tchunk(qtf, c),
                                         start=True, stop=True)
                    nc.vector.scalar_tensor_tensor(
                        stm.rearrange("p (x c) -> p x c", c=128),
                        stp.rearrange("p (x c) -> p x c", c=128), 1.0,
                        masks_g[:, 0:3:2, :], ALU.bypass, ALU.mult)
                    stm_tiles[kc] = (stm[:, :128], None)
                    stm_tiles[nxt] = (stm[:, 128:256], None)
                else:
                    nc.tensor.matmul(stp[:, :128], lhsT=tchunk(ktf, kc),
                                     rhs=tchunk(qtf, kc),
                                     start=True, stop=True)
                    nc.vector.scalar_tensor_tensor(
                        stm[:, :128], stp[:, :128], 1.0, masks[:, 0:128],
                        ALU.bypass, ALU.mult)
                    stm_tiles[kc] = (stm[:, :128], None)

            for i in range(NCH):
                if i not in stm_tiles:
                    emit_scores(i)

                # accumulate num for query chunk qc=i
                ch = plan["chunks"][i]
                slot = num_slot(i)
                kind = ch["kind"]
                nc.tensor.matmul(slot, lhsT=sv_cast(stm_tiles[i][0]),
                                 rhs=sv_cast(vchunk(vsrc, i)),
                                 start=True, stop=False)
                if kind == "W":
                    nc.tensor.matmul(slot,
                                     lhsT=sv_cast(stm_tiles[i - WCH][1]),
                                     rhs=sv_cast(vchunk(vsrc, i - WCH)),
                                     start=False, stop=False)
                    nc.tensor.matmul(slot, lhsT=tchunk(qtf, i),
                                     rhs=sslot(call, i),
                                     start=False, stop=True)
                elif kind == "S":
                    nc.tensor.matmul(slot, lhsT=tchunk(qtf, i),
                                     rhs=sslot(call, i),
                                     start=False, stop=True)
                else:  # G
                    nc.tensor.matmul(slot, lhsT=tchunk(qtf, i),
                                     rhs=sslot(sall, i),
                                     start=False, stop=True)
                    terms = []
                    if ch["alpha"].any():
                        bw = pst.tile([C, M1], F32, tag="st", name=f"bw{i}")
                        nc.tensor.matmul(
                            bw, lhsT=sv_cast(stm_tiles[i - WCH][1]),
                            rhs=sv_cast(vchunk(vsrc, i - WCH)),
                            start=True, stop=False)
                        nc.tensor.matmul(bw, lhsT=tchunk(qtf, i),
                                         rhs=sslot(sall, i - WCH),
                                         start=False, stop=True)
                        terms.append((bw, nalpha[:, i:i + 1]))
                    for sid, _ in ch["groups"]:
                        gp = pst.tile([C, M1], F32, tag="st",
                                      name=f"gp{i}_{sid}")
                        nc.tensor.matmul(gp, lhsT=tchunk(qtf, i),
                                         rhs=sslot(pall, sid),
                                         start=True, stop=True)
                        terms.append((gp, nbeta[:, i * nb + sid:
                                                i * nb + sid + 1]))
                    # fold: slot = main + sum(term * negscale).
                    # DVE reads at most one PSUM operand per op, so move the
                    # main accumulator to SBUF first, then chain terms.
                    acc = tmp_pool.tile([C, M1], F32, tag="gt",
                                        name=f"gacc{i}")
                    nc.scalar.copy(acc, slot)
                    for t_idx, (tps, sc) in enumerate(terms):
                        last = t_idx == len(terms) - 1
                        dst = slot if last else tmp_pool.tile(
                            [C, M1], F32, tag="gt", name=f"gt{i}_{t_idx}")
                        nc.vector.scalar_tensor_tensor(
                            dst, tps, sc, acc, ALU.mult, ALU.add)
                        acc = dst

                # dens for a completed PSUM bank, emitted inline so they
                # run ahead of the remaining mask ops in DVE's queue
                if i % 7 == 6 or i == NCH - 1:
                    g = i // 7
                    dv = pn[:, g, D:D + 65 * 6 + 1:65]
                    sel = slice(7 * g, 7 * g + 7)
                    nc.vector.tensor_scalar_max(dmax[:, sel], dv, EPS)
                    nc.vector.reciprocal(rall[:, sel], dmax[:, sel])

            # ---- scale + store
            for i in range(NCH):
                ob = out_pool.tile([C, D], F32, tag="ob", name=f"ob{i}")
                nc.scalar.activation(ob, num_slot(i)[:, :D], AF.Copy,
                                     scale=rall[:, i:i + 1])
                eng = nc.sync if i % 2 == 0 else nc.scalar
                eng.dma_start(out=d_out[i * C:(i + 1) * C, :], in_=ob)
    return nc


def split_waits(bir: bytes) -> bytes:
    """Walrus codegen caps sync waits at 1 per instruction (2 for
    EventSemaphore); Tile sometimes attaches more.  Hoist the excess into
    preceding same-engine NoOps (engines are in-order, so semantics hold)."""
    import json
    m = json.loads(bir)
    for f in m["functions"]:
        for bb in f["blocks"]:
            out = []
            for ins in bb["instructions"]:
                si = ins.get("sync_info")
                ow = (si or {}).get("on_wait") or []
                cap = 2 if ins.get("opcode") == "EventSemaphore" else 1
                eng = ins.get("engine")
                if eng and len(ow) > cap:
                    keep = ow[-cap:]
                    for j, w in enumerate(ow[:-cap]):
                        out.append({"name": f'{ins["name"]}_sw{j}',
                                    "opcode": "NoOp", "engine": eng,
                                    "ins": [], "outs": [],
                                    "sync_info": {"on_wait": [w],
                                                  "on_update": []}})
                    ins = dict(ins)
                    ins["sync_info"] = {
                        "on_wait": keep,
                        "on_update": (si or {}).get("on_update") or []}
                out.append(ins)
            bb["instructions"] = out
    return json.dumps(m).encode()


# ===================================================== fast path
TH = T // 2             # 2048, packed free dim
F8 = mybir.dt.float8e4
QS = 1.0 / 16.0         # uniform q-feature scale; cancels in num/den
FP8_STM = True
FP8_KTF = True
FP8_QTF = True


def fast_plan(seqlens, win):
    """Return list of segment chunk-ranges if the fast path applies, else None."""
    s = np.asarray(seqlens).astype(np.int64)
    s = np.clip(s, 0, T)
    b = np.unique(np.concatenate([[0], s, [T]]))
    if b[0] != 0 or b[-1] != T:
        return None
    if (b % C).any():
        return None
    segs = []
    for a, e in zip(b[:-1], b[1:]):
        if e - a > win:          # sliding window would activate
            return None
        ca, ce = int(a) // C, int(e) // C
        if ca < NCH // 2 < ce:
            return None          # segment straddles the packing boundary
        segs.append((ca, ce))
    return segs


def pack_head_fast(q, k, v, segs):
    """q,k,v: [T, D] fp32 -> (in1, k2, q2, qx) device layouts (f16)."""
    kn = k.reshape(NCH, C, D).transpose(1, 0, 2).reshape(C, NCH * D)
    va = np.concatenate([v.reshape(NCH, C, D),
                         np.ones((NCH, C, 1), np.float32)], axis=2)
    vaug = va.transpose(1, 0, 2).reshape(C, NCH * M1)
    in1 = np.concatenate([kn, vaug], axis=1).astype(np.float16)
    k2 = k.T.reshape(D, 2, TH).transpose(1, 0, 2).reshape(2 * D, TH)
    q2 = q.T.reshape(D, 2, TH).transpose(1, 0, 2).reshape(2 * D, TH)
    # boundary-query columns: query at t = e*C attends over the whole
    # previous segment (searchsorted side='left' semantics); place its raw
    # q column at the previous segment's partition half.
    bnds = [(a, e) for a, e in segs if e < NCH]
    qx = np.zeros((C, max(1, len(bnds))), np.float32)
    for j, (a, e) in enumerate(bnds):
        hb = D if a >= NCH // 2 else 0
        qx[hb:hb + D, j] = q[e * C, :]
    return (np.ascontiguousarray(in1),
            np.ascontiguousarray(k2.astype(np.float16)),
            np.ascontiguousarray(q2.astype(np.float16)),
            np.ascontiguousarray(qx.astype(np.float16)))


def unpack_out_fast(res):
    """[128, NCH*64] f16 chunk-major -> [T, D] f32"""
    o = np.asarray(res, np.float32).reshape(C, NCH, D)
    return o.transpose(1, 0, 2).reshape(T, D)


def build_bass_fast(segs):
    bnds = [(a, e) for a, e in segs if e < NCH]
    NB = max(1, len(bnds))
    canonical = segs == [(0, 8), (8, 16), (16, 24), (24, 32)]
    nc = bass.Bass()
    d_in1 = nc.dram_tensor("in1", [C, NCH * D + NCH * M1], F16,
                           kind="ExternalInput")
    d_k2 = nc.dram_tensor("k2", [C, TH], F16, kind="ExternalInput")
    d_q2 = nc.dram_tensor("q2", [C, TH], F16, kind="ExternalInput")
    d_qx = nc.dram_tensor("qx", [C, NB], F16, kind="ExternalInput")
    d_masks = nc.dram_tensor("masks", [C, 512], F16, kind="ExternalInput")
    d_out = nc.dram_tensor("out", [C, NCH * D], F16, kind="ExternalOutput")

    seg_start = set(a for a, e in segs)
    bnd_of = {e: (j, a) for j, (a, e) in enumerate(bnds)}
    seg_idx = {}
    for si, (a, e) in enumerate(segs):
        for c in range(a, e):
            seg_idx[c] = (si, a)

    HC = NCH // 2        # chunks per packing half
    HT = TH // 2         # 1024: free-dim half of the packed tensors

    def tchunk(t, j):
        h, r = divmod(j, HC)
        return t[h * D:(h + 1) * D, r * C:(r + 1) * C]

    def kchunk(t, c):
        return t[:, c * D:(c + 1) * D]

    with tile.TileContext(nc) as tc:
        with (
            tc.tile_pool(name="persist", bufs=1) as pp,
            tc.tile_pool(name="stm", bufs=4) as stm_pool,
            tc.tile_pool(name="stm16", bufs=2) as stm16_pool,
            tc.tile_pool(name="pst", bufs=2, space="PSUM") as pst,
            tc.tile_pool(name="pnump", bufs=2, space="PSUM") as pnump,
            tc.tile_pool(name="pgbig", bufs=1, space="PSUM") as pgbig,
        ):
            in1 = pp.tile([C, NCH * D + NCH * M1], F16)
            kn = in1[:, :NCH * D]
            vaug = in1[:, NCH * D:]
            k2 = pp.tile([C, TH], F16)
            q2 = pp.tile([C, TH], F16)
            kf = pp.tile([C, NCH * D], F16)
            ktf = pp.tile([C, TH], F16)
            qtf = pp.tile([C, TH], F16)
            rel_kn = pp.tile([C, NCH * D], F16)
            ktf8 = pp.tile([C, TH], F8, name="ktf8") if FP8_KTF else None
            qtf8 = pp.tile([C, TH], F8, name="qtf8") if FP8_QTF else None
            rel_k = pp.tile([C, TH], F16)
            rel_q = pp.tile([C, TH], F16)
            masks = pp.tile([C, 512], F16)
            rall = pp.tile([C, NCH], F32)
            dmax = pp.tile([C, NCH], F32)
            outall = pp.tile([C, NCH * D], F16)
            qx = pp.tile([C, NB], F16)
            qxf = pp.tile([C, NB], F16)
            qxr = pp.tile([C, NB], F16)
            ps3 = pp.tile([C, NB * M1], F16)
            tiny = pp.tile([1, 1], F32)
            # states: canonical = 7 steps x (2 segs x 65) per half;
            # general = one 65-slot per chunk, half-packed
            sall = pp.tile([C, (7 * 2 if canonical else HC) * M1], F16)

            # pass-1 outer products; bank-safe layout: per half, steps 0-6
            # of seg-even in bank 0, seg-odd in bank 1, segment-final G's in
            # bank 2 (PSUM matmul outputs must not cross a 2KB bank)
            G2 = pgbig.tile([C, 1536], F32)

            def g2slot(c):
                h, r = divmod(c, HC)
                if r % 8 < 7:
                    off = 512 * (r // 8) + M1 * (r % 8)
                else:
                    off = 1024 + M1 * (r // 8)
                return G2[h * D:(h + 1) * D, off:off + M1]

            def sstate(c):
                """SBUF state (exclusive prefix) for chunk c."""
                if canonical:
                    s, j = divmod(c, 8)
                    hb = D * (s // 2)
                    off = (j - 1) * 2 * M1 + (s % 2) * M1
                    return sall[hb:hb + D, off:off + M1]
                h, r = divmod(c, HC)
                return sall[h * D:(h + 1) * D, r * M1:(r + 1) * M1]

            # --- input DMAs first on the scalar HWDGE ring (starts ~3us
            # earlier than the sync ring); outputs use the sync ring
            nc.scalar.dma_start(out=in1, in_=d_in1[:, :])
            nc.scalar.dma_start(out=k2[:, 0:HT], in_=d_k2[:, 0:HT])
            nc.scalar.dma_start(out=q2[:, 0:HT], in_=d_q2[:, 0:HT])
            nc.scalar.dma_start(out=masks, in_=d_masks[:, :])
            nc.scalar.dma_start(out=qx, in_=d_qx[:, :])
            nc.scalar.dma_start(out=k2[:, HT:TH], in_=d_k2[:, HT:TH])
            nc.scalar.dma_start(out=q2[:, HT:TH], in_=d_q2[:, HT:TH])
            # warm the ACT exp table while the DMAs stream
            nc.vector.memset(tiny, 0.0)
            nc.scalar.activation(tiny, tiny, AF.Exp, scale=1.0)

            # --- features: f(x) = min(exp(s x),1) + max(s x,0)
            # exp on ACT; max/min (4x) + add (2x) on DVE
            def feat_exp(dst, src, scale):
                nc.scalar.activation(dst, src, AF.Exp, scale=scale)

            def feat_rest(dst, src, rel, scale, post=1.0):
                if scale == 1.0 and post == 1.0:
                    nc.vector.tensor_scalar(rel, src, 0.0, None, ALU.max)
                else:
                    nc.vector.tensor_scalar(rel, src, 0.0, scale * post,
                                            ALU.max, ALU.mult)
                if post == 1.0:
                    nc.vector.tensor_scalar_min(dst, dst, 1.0)
                else:
                    nc.vector.tensor_scalar(dst, dst, 1.0, post,
                                            ALU.min, ALU.mult)
                nc.vector.tensor_add(dst, dst, rel)

            feat_exp(kf, kn, 1.0)
            feat_exp(ktf[:, 0:HT], k2[:, 0:HT], 1.0)
            feat_exp(qtf[:, 0:HT], q2[:, 0:HT], SCALE)
            feat_exp(ktf[:, HT:TH], k2[:, HT:TH], 1.0)
            feat_exp(qtf[:, HT:TH], q2[:, HT:TH], SCALE)
            feat_exp(qxf, qx, SCALE)

            feat_rest(kf, kn, rel_kn, 1.0)
            feat_rest(ktf[:, 0:HT], k2[:, 0:HT], rel_k[:, 0:HT], 1.0)
            feat_rest(qtf[:, 0:HT], q2[:, 0:HT], rel_q[:, 0:HT], SCALE,
                      QS)

            # --- pass 1: independent outer products into G2
            used = [c for c in range(NCH - 1)
                    if (c + 1 not in seg_start) or (c + 1 in bnd_of)]
            for c in used:
                nc.tensor.matmul(g2slot(c), lhsT=kchunk(kf, c),
                                 rhs=vchunk_of(vaug, c), start=True,
                                 stop=True)

            # --- prefix scan -> SBUF states
            G2r = G2.rearrange("p (b s) -> p b s", s=512)
            if canonical:
                sv = sall.rearrange("p (s c) -> p s c", c=M1)
                for j in range(1, 8):
                    dst = sv[:, 2 * (j - 1):2 * j, :]
                    gsrc = G2r[:, 0:2, M1 * (j - 1):M1 * j]
                    if j == 1:
                        nc.vector.tensor_copy(dst, gsrc)
                    else:
                        nc.vector.scalar_tensor_tensor(
                            dst, gsrc, 0.0,
                            sv[:, 2 * (j - 2):2 * (j - 1), :],
                            ALU.bypass, ALU.add)
            else:
                for a, e in segs:
                    for c in range(a + 1, e):
                        if c == a + 1:
                            nc.vector.tensor_copy(sstate(c), g2slot(c - 1))
                        else:
                            nc.vector.scalar_tensor_tensor(
                                sstate(c), g2slot(c - 1), 0.0,
                                sstate(c - 1), ALU.bypass, ALU.add)
            # full previous-segment sums for the boundary queries
            for a, e in segs:
                if e in bnd_of:
                    j, _ = bnd_of[e]
                    hb = D * (a >= HC)
                    dst = ps3[hb:hb + D, j * M1:(j + 1) * M1]
                    if e - 1 == a:
                        nc.vector.tensor_copy(dst, g2slot(e - 1))
                    else:
                        nc.vector.scalar_tensor_tensor(
                            dst, g2slot(e - 1), 0.0, sstate(e - 1),
                            ALU.bypass, ALU.add)

            if FP8_KTF:
                nc.vector.tensor_copy(ktf8[:, 0:HT], ktf[:, 0:HT])
            if FP8_QTF:
                nc.vector.tensor_copy(qtf8[:, 0:HT], qtf[:, 0:HT])

            # remaining features (second halves + boundary queries)
            feat_rest(ktf[:, HT:TH], k2[:, HT:TH], rel_k[:, HT:TH], 1.0)
            feat_rest(qtf[:, HT:TH], q2[:, HT:TH], rel_q[:, HT:TH], SCALE,
                      QS)
            if FP8_KTF:
                nc.vector.tensor_copy(ktf8[:, HT:TH], ktf[:, HT:TH])
            if FP8_QTF:
                nc.vector.tensor_copy(qtf8[:, HT:TH], qtf[:, HT:TH])
            nc.vector.tensor_scalar(qxr, qx, 0.0, SCALE * QS,
                                    ALU.max, ALU.mult)
            nc.vector.tensor_scalar(qxf, qxf, 1.0, QS, ALU.min, ALU.mult)
            nc.vector.tensor_add(qxf, qxf, qxr)

            # --- pass 2
            masks_q = masks.rearrange("p (x c) -> p x c", c=128)
            stm_tiles = {}
            gtiles = {}
            shipped = [0]

            def emit_quad(qd):
                c0 = qd * 4
                ncn = min(4, NCH - c0)
                stp = pst.tile([C, 512], F32, tag="st", name=f"stp{qd}")
                stm = stm_pool.tile([C, 512], F8 if FP8_STM else F16,
                                    tag="stm", name=f"stm{qd}")
                starts = [x for x in range(ncn) if c0 + x in seg_start]
                stm16 = None
                if FP8_STM and starts:
                    stm16 = stm16_pool.tile([C, 512], F16, tag="stm16",
                                            name=f"stm16_{qd}")
                for x in range(ncn):
                    cc = c0 + x
                    nc.tensor.matmul(stp[:, 128 * x:128 * (x + 1)],
                                     lhsT=tchunk(ktf8 if FP8_KTF else ktf,
                                                 cc),
                                     rhs=tchunk(qtf, cc),
                                     start=True, stop=True)

                def mask_run(x0, x1, dst):
                    nn0, nn1 = 128 * x0, 128 * x1
                    nc.vector.scalar_tensor_tensor(
                        dst[:, nn0:nn1].rearrange("p (x c) -> p x c",
                                                  c=128),
                        stp[:, nn0:nn1].rearrange("p (x c) -> p x c",
                                                  c=128), 1.0,
                        masks_q[:, 0:1, :].broadcast_to([C, x1 - x0, 128]),
                        ALU.bypass, ALU.mult)
                    for x in range(x0, x1):
                        stm_tiles[c0 + x] = dst[:, 128 * x:128 * (x + 1)]

                if stm16 is None:
                    mask_run(0, ncn, stm)
                else:
                    # segment-start chunks keep f16 scores (small den);
                    # the rest go fp8
                    x = 0
                    while x < ncn:
                        f16_run = (c0 + x) in seg_start
                        x1 = x + 1
                        while (x1 < ncn
                               and ((c0 + x1) in seg_start) == f16_run):
                            x1 += 1
                        mask_run(x, x1, stm16 if f16_run else stm)
                        x = x1

            emit_quad(0)
            emit_quad(1)

            for c in range(NCH):
                g = c // 7
                if c % 7 == 0:
                    gtiles[g] = pnump.tile([C, 512], F32, tag="num",
                                           name=f"num{g}")
                slot = gtiles[g][:, (c - 7 * g) * M1:(c - 7 * g + 1) * M1]
                first = c in seg_start
                bnd = bnd_of.get(c)
                nc.tensor.matmul(slot, lhsT=stm_tiles[c],
                                 rhs=vchunk_of(vaug, c),
                                 start=True, stop=first and bnd is None)
                if not first:
                    nc.tensor.matmul(slot,
                                     lhsT=tchunk(qtf8 if FP8_QTF else qtf,
                                                 c),
                                     rhs=sstate(c), start=False, stop=True)
                elif bnd is not None:
                    j, a_prev = bnd
                    hb = D * (a_prev >= HC)
                    nc.tensor.matmul(slot[0:1, :],
                                     lhsT=qxf[hb:hb + D, j:j + 1],
                                     rhs=ps3[hb:hb + D,
                                             j * M1:(j + 1) * M1],
                                     start=False, stop=True,
                                     skip_group_check=True)
                if c % 4 == 3 and (c // 4 + 2) * 4 < NCH:
                    emit_quad(c // 4 + 2)

                if c % 7 == 6 or c == NCH - 1:
                    n = c - 7 * g + 1
                    gt = gtiles[g]
                    dv = gt[:, D:D + M1 * (n - 1) + 1:M1]
                    sel = slice(7 * g, 7 * g + n)
                    nc.vector.tensor_scalar_max(dmax[:, sel], dv, EPS)
                    nc.vector.reciprocal(rall[:, sel], dmax[:, sel])
                    # batched scale: out[p,i,m] = num[p,i*65+m] * rall[p,i]
                    pv = gt[:, 0:455].rearrange("p (i m) -> p i m",
                                                m=M1)[:, 0:n, 0:D]
                    rv = rall[:, sel].rearrange(
                        "p (i u) -> p i u", u=1).broadcast_to([C, n, D])
                    ov = outall[:, 7 * g * D:(7 * g + n) * D].rearrange(
                        "p (i m) -> p i m", m=D)
                    nc.vector.tensor_tensor(ov, pv, rv, ALU.mult)
                    done = ((c + 1) // 8) * 8 if c < NCH - 1 else NCH
                    lo, hi = shipped[0] * D, done * D
                    if hi > lo:
                        nc.sync.dma_start(out=d_out[:, lo:hi],
                                          in_=outall[:, lo:hi])
                        shipped[0] = done
    return nc


def vchunk_of(vaug, c):
    return vaug[:, c * M1:(c + 1) * M1]


# ------------------------------------------------------------------ driver
def _build_masks_fast():
    tri_a = np.triu(np.ones((C, C), np.float32))
    masks = np.zeros((C, 512), np.float32)
    masks[:, 0:128] = tri_a
    masks[:, 256:384] = tri_a
    return masks.astype(np.float16)


def kernel(**inputs):
    q = np.ascontiguousarray(np.asarray(inputs["q"]), dtype=np.float32)
    k = np.ascontiguousarray(np.asarray(inputs["k"]), dtype=np.float32)
    v = np.ascontiguousarray(np.asarray(inputs["v"]), dtype=np.float32)
    seqlens = np.asarray(inputs["seqlens"])
    assert q.shape == (T, H, D), q.shape

    segs = fast_plan(seqlens, WIN)
    if segs is not None:
        nc = build_bass_fast(segs)
        patched = split_waits(nc.to_json_bytes())
        nc.to_json_bytes = lambda: patched
        masks = _build_masks_fast()
        in_maps = []
        for h in range(H):
            in1, k2, q2, qx = pack_head_fast(q[:, h], k[:, h], v[:, h],
                                             segs)
            in_maps.append(dict(in1=in1, k2=k2, q2=q2, qx=qx, masks=masks))
        res = run_bass_kernel_spmd(nc, in_maps, core_ids=list(range(H)),
                                   trace=TRACE)
        if TRACE:
            kernel.last_result = res
        out = np.empty((T, H, D), np.float32)
        for h in range(H):
            out[:, h, :] = unpack_out_fast(res.results[h]["out"])
        return out

    plan = host_plan(seqlens)
    masks, pmask, negalpha, negbeta = build_aux(plan)
    nc = build_bass(plan)
    patched = split_waits(nc.to_json_bytes())
    nc.to_json_bytes = lambda: patched

    in_maps = []
    for h in range(H):
        qtp, ktp, kn, vaug = pack_head(q[:, h], k[:, h], v[:, h])
        im = dict(qtp=qtp, ktp=ktp, kn=kn, vaug=vaug,
                  masks=masks.astype(np.float16),
                  pmask=pmask, negalpha=negalpha, negbeta=negbeta)
        in_maps.append(im)

    res = run_bass_kernel_spmd(nc, in_maps, core_ids=list(range(H)),
                               trace=TRACE)
    if TRACE:
        kernel.last_result = res
    out = np.empty((T, H, D), np.float32)
    for h in range(H):
        out[:, h, :] = res.results[h]["out"]
    return out
